# revision 11
# baseline (speedup 1.0000x reference)
"""Trainium2 Bass/Tile kernel for nn_FB_FMM (sparse_attention).

Computation (per batch element b, with N = H*W = 4096 tokens, C=256, D=32):
  1. Self-attention:  sa_out = attn(conv(x,sa_wq), conv(x,sa_wk), conv(x,sa_wv))
     x' = sa_gamma * sa_out + x
  2. Masked cross-attention (FB_FMM):
     ff = mask * x'; fb = (1-mask) * x'
     sw_bg = attn(conv(ff,wq), conv(fb,wk), conv(fb,wv))
     out = x' + gamma * ff * (std(sw_bg)/std(ff))    [per-channel std, ddof=1]

Sharding: 8 cores = 2 batch groups x 4-way query-row sharding (1024 rows each).
Each core computes its row-chunk of both attention layers; K/V sides are
computed redundantly per core (cheap: D=32 / one C x C conv). Cross-core
communication inside the kernel:
  - AllGather of x' chunks within each 4-core batch group (layer-2 K/V need
    the full x'), split into two 512-row phases so the first overlaps the
    second half of the layer-1 attention loop.
  - AllReduce of per-channel [sum, sumsq] stats for the FMM std ratio.

Layouts: feature maps are channel-major (C on partitions). Scores are computed
transposed (S^T: keys j on partitions, queries i free; logits are small so exp
needs no max-subtraction pass). The AV matmul keeps V^T slices stationary
(weight reuse) with E^T moving, producing O in natural (c x i) layout; the
softmax denominator comes from one extra M=1 ones-matmul per tile, and the
reciprocal row is broadcast across partitions with a K=1 ones matmul. All
heavy matmuls run in float32r (1 cycle/row vs 4 for fp32; ~1e-4 relative
rounding). V-conv biases are folded out mathematically (sum_j A[i,j] = 1
makes the layer-1 V bias a constant shift folded into the residual; variance
is shift-invariant so the layer-2 V bias drops out of the FMM std).
"""

import numpy as np

P = 128
B, C, HH, WW = 2, 256, 64, 64
N = HH * WW            # 4096 tokens
D = 32                 # q/k channels
NCORES = 8
RSH = 4                # row shards per batch group
R = N // RSH           # 1024 query rows per core
NT = N // P            # 32 key tiles
IC = 512               # query i-chunk (one PSUM bank of fp32)
EPS = 1e-5
F32 = np.float32

_CACHE = {}


def _build_bass():
    """Build the Bass/Tile program (single SPMD NEFF for all 8 cores)."""
    import concourse.bass as bass
    from concourse import bacc, mybir, tile

    f32 = mybir.dt.float32
    f32r = mybir.dt.float32r
    bf16 = mybir.dt.bfloat16
    AX = mybir.AxisListType
    OP = mybir.AluOpType
    AF = mybir.ActivationFunctionType

    nc = bacc.Bacc(
        "TRN2", target_bir_lowering=False, debug=False, num_devices=NCORES
    )

    # ---------------- I/O ----------------
    xf_d = nc.dram_tensor("xf", [C, N], f32r, kind="ExternalInput")
    xc_d = nc.dram_tensor("xc", [C, R], f32r, kind="ExternalInput")
    mrow_d = nc.dram_tensor("mrow", [1, N], f32, kind="ExternalInput")
    mcrow_d = nc.dram_tensor("mcrow", [1, R], f32, kind="ExternalInput")
    wqT1_d = nc.dram_tensor("wqT1", [C, D], f32r, kind="ExternalInput")
    wkT1_d = nc.dram_tensor("wkT1", [C, D], f32r, kind="ExternalInput")
    wvT1_d = nc.dram_tensor("wvT1", [C, C], f32r, kind="ExternalInput")
    wqT2_d = nc.dram_tensor("wqT2", [C, D], f32r, kind="ExternalInput")
    wkT2_d = nc.dram_tensor("wkT2", [C, D], f32r, kind="ExternalInput")
    wvT2_d = nc.dram_tensor("wvT2", [C, C], f32r, kind="ExternalInput")
    # consts columns: 0 sa_gamma, 1 gamma, 2/3 sa_gamma*sa_bv halves,
    # 6 sa_bq, 7 sa_bk, 8 bq, 9 bk (cols 6-9 live on partitions 0..31)
    consts_d = nc.dram_tensor("consts", [P, 10], f32, kind="ExternalInput")
    out_d = nc.dram_tensor("outc", [C, R], f32, kind="ExternalOutput")

    groups = [[0, 1, 2, 3], [4, 5, 6, 7]]

    with tile.TileContext(nc) as tc:
        from contextlib import ExitStack

        ctx = ExitStack()
        with ctx:
            big = ctx.enter_context(tc.tile_pool(name="big", bufs=1))
            epool = ctx.enter_context(tc.tile_pool(name="epool", bufs=3))
            onpool = ctx.enter_context(tc.tile_pool(name="onpool", bufs=3))
            sqpool = ctx.enter_context(tc.tile_pool(name="sqpool", bufs=2))
            fbpool = ctx.enter_context(tc.tile_pool(name="fbpool", bufs=4))
            rcpool = ctx.enter_context(tc.tile_pool(name="rcpool", bufs=4))
            finpool = ctx.enter_context(tc.tile_pool(name="finpool", bufs=2))
            misc = ctx.enter_context(tc.tile_pool(name="misc", bufs=1))
            psA = ctx.enter_context(
                tc.tile_pool(name="psA", bufs=2, space="PSUM")
            )
            psS = ctx.enter_context(
                tc.tile_pool(name="psS", bufs=2, space="PSUM")
            )
            psO = ctx.enter_context(
                tc.tile_pool(name="psO", bufs=4, space="PSUM")
            )
            dram = ctx.enter_context(
                tc.tile_pool(name="dram", bufs=1, space="DRAM")
            )

            # ------------- persistent SBUF tiles -------------
            xc_sb = big.tile([P, 2, R], f32r, tag="xc", name="xc_sb")
            mask_sb = big.tile([P, N], f32, tag="mask", name="mask_sb")
            maskc_sb = big.tile([P, R], f32, tag="maskc", name="maskc_sb")
            xp_sb = big.tile([P, 2, R], f32, tag="xp", name="xp_sb")
            ff_sb = big.tile([P, 2, R], f32r, tag="ff", name="ff_sb")
            wqT1_sb = big.tile([P, 2, D], f32r, tag="wqT1", name="wqT1_sb")
            wkT1_sb = big.tile([P, 2, D], f32r, tag="wkT1", name="wkT1_sb")
            wvT1_sb = big.tile([P, 2, C], f32r, tag="wvT1", name="wvT1_sb")
            wqT2_sb = big.tile([P, 2, D], f32r, tag="wqT2", name="wqT2_sb")
            wkT2_sb = big.tile([P, 2, D], f32r, tag="wkT2", name="wkT2_sb")
            wvT2_sb = big.tile([P, 2, C], f32r, tag="wvT2", name="wvT2_sb")
            consts_sb = big.tile([P, 10], f32, tag="consts", name="consts_sb")
            # ones column (f32r) for the denominator matmul; ones row (f32)
            # for the K=1 reciprocal-replication matmul
            onesc_sb = big.tile([P, 1], bf16, tag="onesc", name="onesc_sb")
            onesr_sb = big.tile([1, P], f32, tag="onesr", name="onesr_sb")
            stats_sb = misc.tile([P, 8], f32, tag="stats", name="stats_sb")

            # input DMAs
            nc.sync.dma_start(out=consts_sb[:], in_=consts_d[:])
            for k in range(2):
                cs = slice(k * P, (k + 1) * P)
                nc.sync.dma_start(out=wqT1_sb[:, k, :], in_=wqT1_d[cs, :])
                nc.sync.dma_start(out=wkT1_sb[:, k, :], in_=wkT1_d[cs, :])
                nc.sync.dma_start(out=wvT1_sb[:, k, :], in_=wvT1_d[cs, :])
                nc.sync.dma_start(out=wqT2_sb[:, k, :], in_=wqT2_d[cs, :])
                nc.sync.dma_start(out=wkT2_sb[:, k, :], in_=wkT2_d[cs, :])
                nc.sync.dma_start(out=wvT2_sb[:, k, :], in_=wvT2_d[cs, :])
                nc.sync.dma_start(out=xc_sb[:, k, :], in_=xc_d[cs, :])
            nc.sync.dma_start(
                out=mask_sb[:], in_=mrow_d[0, :].partition_broadcast(P)
            )
            nc.sync.dma_start(
                out=maskc_sb[:], in_=mcrow_d[0, :].partition_broadcast(P)
            )
            nc.vector.memset(onesc_sb[:], 1.0)
            nc.vector.memset(onesr_sb[:], 1.0)

            # x_full: chunked DMA so layer-1 convs can start early.
            xf_sb = big.tile([P, 2, N], f32r, tag="xbig", name="xf_sb")
            for k in range(2):
                for jc in range(4):
                    js = slice(jc * (N // 4), (jc + 1) * (N // 4))
                    nc.sync.dma_start(
                        out=xf_sb[:, k, js],
                        in_=xf_d[k * P : (k + 1) * P, js],
                    )

            def conv_qk(wT_sb, bias_col, src_of, width, out_sb):
                """out (D x width) = wT.T @ src + bias.  src_of(k, js) gives
                the (128 x 512) input-channel tile."""
                for jc in range(width // IC):
                    js = slice(jc * IC, (jc + 1) * IC)
                    ps = psA.tile([D, IC], f32, tag="a", name="qk_ps")
                    nc.tensor.matmul(
                        ps[:], wT_sb[:, 0, :], src_of(0, js),
                        start=True, stop=False,
                    )
                    nc.tensor.matmul(
                        ps[:], wT_sb[:, 1, :], src_of(1, js),
                        start=False, stop=True,
                    )
                    nc.vector.tensor_scalar_add(
                        out_sb[:, js], ps[:],
                        consts_sb[0:D, bias_col : bias_col + 1],
                    )

            def conv_vT(wvT_sb, src_of, v_sb, t):
                """v_sb[:, t, :] = (src^T @ wvT) for key tile t (j on
                partitions, channels free)."""
                ts_ = slice(t * P, (t + 1) * P)
                ps = psA.tile([P, C], f32, tag="a", name="v_ps")
                nc.tensor.matmul(
                    ps[:], src_of(0, ts_), wvT_sb[:, 0, :],
                    start=True, stop=False,
                )
                nc.tensor.matmul(
                    ps[:], src_of(1, ts_), wvT_sb[:, 1, :],
                    start=False, stop=True,
                )
                nc.vector.tensor_copy(v_sb[:, t, :], ps[:])

            def attention(q_sb, k_sb, v_sb, epilogue):
                """Row-chunk attention.  Per i-chunk: S^T = K-tile^T Q
                (j on partitions), E = exp(S^T), then O(c,i) accumulates
                with V^T slices stationary and E moving; the denominator
                row comes from an M=1 ones matmul.  epilogue(ich, accs,
                rrep) gets natural-layout unnormalized O accumulators and
                the partition-replicated reciprocal denominator (PSUM)."""
                for ich in range(R // IC):
                    is_ = slice(ich * IC, (ich + 1) * IC)
                    accs = [
                        psO.tile([P, IC], f32, tag="o", name="acc")
                        for _ in range(2)
                    ]
                    den = psA.tile([1, IC], f32, tag="a", name="den")
                    for t in range(NT):
                        sps = psS.tile([P, IC], f32, tag="s", name="s_ps")
                        nc.tensor.matmul(
                            sps[:],
                            k_sb[:, t * P : (t + 1) * P],
                            q_sb[:, is_],
                            start=True, stop=True,
                        )
                        e_sb = epool.tile([P, IC], bf16, tag="e", name="e_sb")
                        nc.scalar.activation(e_sb[:], sps[:], AF.Exp)
                        for ct in range(2):
                            nc.tensor.matmul(
                                accs[ct][:],
                                v_sb[:, t, ct * P : (ct + 1) * P],
                                e_sb[:],
                                start=(t == 0), stop=(t == NT - 1),
                            )
                        nc.tensor.matmul(
                            den[:], onesc_sb[:], e_sb[:],
                            start=(t == 0), stop=(t == NT - 1),
                        )
                    # reciprocal of the denominator row, replicated to all
                    # partitions via a K=1 ones matmul
                    rrow = rcpool.tile([1, IC], f32, tag="rc", name="rrow")
                    nc.vector.reciprocal(rrow[:], den[:])
                    rrep_ps = psA.tile([P, IC], f32, tag="a", name="rrep_ps")
                    nc.tensor.matmul(
                        rrep_ps[:], onesr_sb[:], rrow[:], start=True, stop=True
                    )
                    rrep = onpool.tile([P, IC], f32, tag="rr", name="rrep")
                    nc.scalar.copy(rrep[:], rrep_ps[:])
                    epilogue(ich, accs, rrep)

            # ================= Layer 1: self-attention =================
            q1_sb = big.tile([D, R], f32r, tag="q", name="q1_sb")
            k1_sb = big.tile([D, N], f32r, tag="k", name="k1_sb")
            v1_sb = big.tile([P, NT, C], bf16, tag="v", name="v1_sb")

            conv_qk(wqT1_sb, 6, lambda k, js: xc_sb[:, k, js], R, q1_sb)
            conv_qk(wkT1_sb, 7, lambda k, js: xf_sb[:, k, js], N, k1_sb)
            for t in range(NT):
                conv_vT(wvT1_sb, lambda k, ts_: xf_sb[:, k, ts_], v1_sb, t)

            def epilogue1(ich, accs, rrep):
                io = slice(ich * IC, (ich + 1) * IC)
                for ct in range(2):
                    # x' = sa_gamma * (O/den) + sa_gamma*bv + x
                    nc.vector.tensor_mul(
                        xp_sb[:, ct, io], accs[ct][:], rrep[:]
                    )
                    nc.vector.tensor_scalar(
                        xp_sb[:, ct, io], xp_sb[:, ct, io],
                        consts_sb[:, 0:1], consts_sb[:, 2 + ct : 3 + ct],
                        op0=OP.mult, op1=OP.add,
                    )
                    nc.vector.tensor_add(
                        xp_sb[:, ct, io],
                        xp_sb[:, ct, io],
                        xc_sb[:, ct, io].bitcast(f32),
                    )

            attention(q1_sb, k1_sb, v1_sb, epilogue1)

            # ====== AllGather x' within each batch group (2 phases) ======
            # Phase h gathers x' columns [h*512, (h+1)*512) of every rank;
            # phase 0 overlaps the second layer-1 attention i-chunk.
            xpf_sb = big.tile([P, 2, N], f32, tag="xbig", name="xpf_sb")
            for h in range(2):
                hs = slice(h * IC, (h + 1) * IC)
                ag_in = dram.tile(
                    [C, IC], f32, tag=f"ag_in{h}", name=f"ag_in{h}"
                )
                ag_out = dram.tile(
                    [RSH, C, IC], f32, tag=f"ag_out{h}", name=f"ag_out{h}"
                )
                for ct in range(2):
                    nc.sync.dma_start(
                        out=ag_in[ct * P : (ct + 1) * P, :],
                        in_=xp_sb[:, ct, hs],
                    )
                nc.gpsimd.collective_compute(
                    "AllGather",
                    OP.bypass,
                    replica_groups=groups,
                    ins=[ag_in[:].opt()],
                    outs=[ag_out[:].opt()],
                )
                for ct in range(2):
                    for r in range(RSH):
                        nc.sync.dma_start(
                            out=xpf_sb[
                                :, ct, r * R + h * IC : r * R + (h + 1) * IC
                            ],
                            in_=ag_out[r, ct * P : (ct + 1) * P, :],
                        )

            # ============== Layer 2: masked cross-attention ==============
            # feature_f chunk + its per-channel stats (cols 0-3 of stats_sb)
            ffsq = misc.tile([P, R], f32, tag="ffsq", name="ffsq")
            for ct in range(2):
                nc.vector.tensor_mul(
                    ff_sb[:, ct, :], maskc_sb[:], xp_sb[:, ct, :]
                )
                nc.vector.tensor_reduce(
                    stats_sb[:, ct : ct + 1], ff_sb[:, ct, :].bitcast(f32),
                    axis=AX.X, op=OP.add,
                )
                nc.vector.tensor_mul(
                    ffsq[:],
                    ff_sb[:, ct, :].bitcast(f32),
                    ff_sb[:, ct, :].bitcast(f32),
                )
                nc.vector.tensor_reduce(
                    stats_sb[:, 2 + ct : 3 + ct], ffsq[:],
                    axis=AX.X, op=OP.add,
                )

            q2_sb = big.tile([D, R], f32r, tag="q", name="q2_sb")
            conv_qk(wqT2_sb, 8, lambda k, js: ff_sb[:, k, js], R, q2_sb)

            # feature_b tiles on the fly -> K2 and V2^T convs.  Chunk order
            # interleaves gather phases: even chunks only need AG phase 0.
            k2_sb = big.tile([D, N], f32r, tag="k", name="k2_sb")
            v2_sb = big.tile([P, NT, C], bf16, tag="v", name="v2_sb")
            for jc in (0, 2, 4, 6, 1, 3, 5, 7):
                js = slice(jc * IC, (jc + 1) * IC)
                fb0 = fbpool.tile([P, IC], f32r, tag="fb", name="fb0")
                fb1 = fbpool.tile([P, IC], f32r, tag="fb", name="fb1")
                # fb = (1-mask)*x' = x' - mask*x'
                nc.vector.tensor_mul(
                    fb0[:], mask_sb[:, js], xpf_sb[:, 0, js]
                )
                nc.vector.tensor_sub(
                    fb0[:], xpf_sb[:, 0, js], fb0[:].bitcast(f32)
                )
                nc.vector.tensor_mul(
                    fb1[:], mask_sb[:, js], xpf_sb[:, 1, js]
                )
                nc.vector.tensor_sub(
                    fb1[:], xpf_sb[:, 1, js], fb1[:].bitcast(f32)
                )
                ps = psA.tile([D, IC], f32, tag="a", name="k2_ps")
                nc.tensor.matmul(
                    ps[:], wkT2_sb[:, 0, :], fb0[:], start=True, stop=False
                )
                nc.tensor.matmul(
                    ps[:], wkT2_sb[:, 1, :], fb1[:], start=False, stop=True
                )
                nc.vector.tensor_scalar_add(
                    k2_sb[:, js], ps[:], consts_sb[0:D, 9:10]
                )
                for tsub in range(IC // P):
                    t = jc * (IC // P) + tsub
                    ts_ = slice(tsub * P, (tsub + 1) * P)
                    psv = psA.tile([P, C], f32, tag="a", name="v2_ps")
                    nc.tensor.matmul(
                        psv[:], fb0[:, ts_], wvT2_sb[:, 0, :],
                        start=True, stop=False,
                    )
                    nc.tensor.matmul(
                        psv[:], fb1[:, ts_], wvT2_sb[:, 1, :],
                        start=False, stop=True,
                    )
                    nc.vector.tensor_copy(v2_sb[:, t, :], psv[:])

            def epilogue2(ich, accs, rrep):
                # normalized sw_bg chunk in natural layout; accumulate
                # per-channel sum/sumsq into stats_sb cols 4-7 via VE
                for ct in range(2):
                    onb = onpool.tile([P, IC], f32, tag="on", name="on2")
                    nc.vector.tensor_mul(onb[:], accs[ct][:], rrep[:])
                    s1 = rcpool.tile([P, 1], f32, tag="s1", name="s1")
                    nc.vector.tensor_reduce(
                        s1[:], onb[:], axis=AX.X, op=OP.add
                    )
                    sqb = sqpool.tile([P, IC], f32, tag="sq", name="sq2")
                    nc.vector.tensor_mul(sqb[:], onb[:], onb[:])
                    s2 = rcpool.tile([P, 1], f32, tag="s2", name="s2")
                    nc.vector.tensor_reduce(
                        s2[:], sqb[:], axis=AX.X, op=OP.add
                    )
                    if ich == 0:
                        nc.vector.tensor_copy(
                            stats_sb[:, 4 + ct : 5 + ct], s1[:]
                        )
                        nc.vector.tensor_copy(
                            stats_sb[:, 6 + ct : 7 + ct], s2[:]
                        )
                    else:
                        nc.vector.tensor_add(
                            stats_sb[:, 4 + ct : 5 + ct],
                            stats_sb[:, 4 + ct : 5 + ct], s1[:],
                        )
                        nc.vector.tensor_add(
                            stats_sb[:, 6 + ct : 7 + ct],
                            stats_sb[:, 6 + ct : 7 + ct], s2[:],
                        )

            attention(q2_sb, k2_sb, v2_sb, epilogue2)

            # ================== stats AllReduce + FMM ==================
            ar_in = dram.tile([P, 8], f32, tag="ar_in", name="ar_in")
            ar_out = dram.tile([P, 8], f32, tag="ar_out", name="ar_out")
            nc.sync.dma_start(out=ar_in[:], in_=stats_sb[:])
            nc.gpsimd.collective_compute(
                "AllReduce",
                OP.add,
                replica_groups=groups,
                ins=[ar_in[:].opt()],
                outs=[ar_out[:].opt()],
            )
            rst = misc.tile([P, 8], f32, tag="rst", name="rst")
            nc.sync.dma_start(out=rst[:], in_=ar_out[:])

            # var = (S2 - S1^2/N)/(N-1) + EPS (both channel-halves at once)
            varf = misc.tile([P, 2], f32, tag="varf", name="varf")
            varg = misc.tile([P, 2], f32, tag="varg", name="varg")
            ratio = misc.tile([P, 2], f32, tag="ratio", name="ratio")
            for var, s1s, s2s in ((varf, 0, 2), (varg, 4, 6)):
                nc.vector.tensor_mul(
                    var[:], rst[:, s1s : s1s + 2], rst[:, s1s : s1s + 2]
                )
                nc.vector.tensor_scalar(
                    var[:], var[:], -1.0 / N, None, op0=OP.mult
                )
                nc.vector.tensor_add(var[:], var[:], rst[:, s2s : s2s + 2])
                nc.vector.tensor_scalar(
                    var[:], var[:], 1.0 / (N - 1), EPS, op0=OP.mult, op1=OP.add
                )
            nc.vector.reciprocal(varf[:], varf[:])
            nc.vector.tensor_mul(varg[:], varg[:], varf[:])
            nc.scalar.activation(ratio[:], varg[:], AF.Sqrt)
            # fold in gamma
            nc.vector.tensor_scalar_mul(ratio[:], ratio[:], consts_sb[:, 1:2])

            # out = x' + (gamma * std_bg/std_f) * ff
            for ct in range(2):
                fin = finpool.tile([P, R], f32, tag="fin", name="fin")
                nc.vector.tensor_scalar_mul(
                    fin[:], ff_sb[:, ct, :].bitcast(f32),
                    ratio[:, ct : ct + 1],
                )
                nc.vector.tensor_add(fin[:], fin[:], xp_sb[:, ct, :])
                nc.sync.dma_start(
                    out=out_d[ct * P : (ct + 1) * P, :], in_=fin[:]
                )

    nc.compile()
    return nc


def _prep_inputs(x, mask, sa_wq, sa_bq, sa_wk, sa_bk, sa_wv, sa_bv, sa_gamma,
                 wq, bq, wk, bk, wv, bv, gamma):
    """Build the per-core input maps (host-side sharding + weight layout)."""
    x = np.ascontiguousarray(x, dtype=F32)
    mask = np.ascontiguousarray(mask, dtype=F32)

    wqT1 = np.ascontiguousarray(sa_wq.T, dtype=F32)
    wkT1 = np.ascontiguousarray(sa_wk.T, dtype=F32)
    wvT1 = np.ascontiguousarray(sa_wv.T, dtype=F32)
    wqT2 = np.ascontiguousarray(wq.T, dtype=F32)
    wkT2 = np.ascontiguousarray(wk.T, dtype=F32)
    wvT2 = np.ascontiguousarray(wv.T, dtype=F32)

    consts = np.zeros((P, 10), dtype=F32)
    consts[:, 0] = sa_gamma[0]
    consts[:, 1] = gamma[0]
    sgb = (sa_gamma[0] * sa_bv).astype(F32)
    consts[:, 2] = sgb[0:P]
    consts[:, 3] = sgb[P:C]
    consts[0:D, 6] = sa_bq
    consts[0:D, 7] = sa_bk
    consts[0:D, 8] = bq
    consts[0:D, 9] = bk

    in_maps = []
    for g in range(NCORES):
        b, r = g // RSH, g % RSH
        xb = np.ascontiguousarray(x[b].reshape(C, N))
        mb = np.ascontiguousarray(mask[b].reshape(1, N))
        in_maps.append({
            "xf": xb,
            "xc": np.ascontiguousarray(xb[:, r * R : (r + 1) * R]),
            "mrow": mb,
            "mcrow": np.ascontiguousarray(mb[:, r * R : (r + 1) * R]),
            "wqT1": wqT1, "wkT1": wkT1, "wvT1": wvT1,
            "wqT2": wqT2, "wkT2": wkT2, "wvT2": wvT2,
            "consts": consts,
        })
    return in_maps


def kernel(**inputs):
    from concourse import bass_utils

    if "nc" not in _CACHE:
        _CACHE["nc"] = _build_bass()
    nc = _CACHE["nc"]

    in_maps = _prep_inputs(**inputs)
    res = bass_utils.run_bass_kernel_spmd(
        nc, in_maps, core_ids=list(range(NCORES))
    )
    _CACHE["last_results"] = res

    out = np.empty((B, C, N), dtype=F32)
    for g in range(NCORES):
        b, r = g // RSH, g % RSH
        out[b, :, r * R : (r + 1) * R] = res.results[g]["outc"]
    return out.reshape(B, C, HH, WW)


# revision 12
# speedup vs baseline: 1.0592x; 1.0592x over previous
"""Trainium2 Bass/Tile kernel for nn_FB_FMM (sparse_attention).

Computation (per batch element b, with N = H*W = 4096 tokens, C=256, D=32):
  1. Self-attention:  sa_out = attn(conv(x,sa_wq), conv(x,sa_wk), conv(x,sa_wv))
     x' = sa_gamma * sa_out + x
  2. Masked cross-attention (FB_FMM):
     ff = mask * x'; fb = (1-mask) * x'
     sw_bg = attn(conv(ff,wq), conv(fb,wk), conv(fb,wv))
     out = x' + gamma * ff * (std(sw_bg)/std(ff))    [per-channel std, ddof=1]

Sharding: 8 cores = 2 batch groups x 4-way query-row sharding (1024 rows each).
Each core computes its row-chunk of both attention layers; K/V sides are
computed redundantly per core (cheap: D=32 / one C x C conv). Cross-core
communication inside the kernel:
  - AllGather of x' chunks within each 4-core batch group (layer-2 K/V need
    the full x'), split into two 512-row phases so the first overlaps the
    second half of the layer-1 attention loop.
  - AllReduce of per-channel [sum, sumsq] stats for the FMM std ratio.

Layouts: feature maps are channel-major (C on partitions). Scores are computed
transposed (S^T: keys j on partitions, queries i free; logits are small so exp
needs no max-subtraction pass). The AV matmul keeps V^T slices stationary
(weight reuse) with E^T moving, producing O in natural (c x i) layout; the
softmax denominator comes from one extra M=1 ones-matmul per tile, and the
reciprocal row is broadcast across partitions with a K=1 ones matmul. All
heavy matmuls run in float32r (1 cycle/row vs 4 for fp32; ~1e-4 relative
rounding). V-conv biases are folded out mathematically (sum_j A[i,j] = 1
makes the layer-1 V bias a constant shift folded into the residual; variance
is shift-invariant so the layer-2 V bias drops out of the FMM std).
"""

import numpy as np

P = 128
B, C, HH, WW = 2, 256, 64, 64
N = HH * WW            # 4096 tokens
D = 32                 # q/k channels
NCORES = 8
RSH = 4                # row shards per batch group
R = N // RSH           # 1024 query rows per core
NT = N // P            # 32 key tiles
IC = 512               # query i-chunk (one PSUM bank of fp32)
EPS = 1e-5
F32 = np.float32

_CACHE = {}


def _build_bass():
    """Build the Bass/Tile program (single SPMD NEFF for all 8 cores)."""
    import concourse.bass as bass
    from concourse import bacc, mybir, tile

    f32 = mybir.dt.float32
    f32r = mybir.dt.float32r
    bf16 = mybir.dt.bfloat16
    AX = mybir.AxisListType
    OP = mybir.AluOpType
    AF = mybir.ActivationFunctionType

    nc = bacc.Bacc(
        "TRN2", target_bir_lowering=False, debug=False, num_devices=NCORES
    )

    # ---------------- I/O ----------------
    xf_d = nc.dram_tensor("xf", [C, N], f32r, kind="ExternalInput")
    xc_d = nc.dram_tensor("xc", [C, R], f32r, kind="ExternalInput")
    mrow_d = nc.dram_tensor("mrow", [1, N], f32, kind="ExternalInput")
    mcrow_d = nc.dram_tensor("mcrow", [1, R], f32, kind="ExternalInput")
    wqT1_d = nc.dram_tensor("wqT1", [C, D], f32r, kind="ExternalInput")
    wkT1_d = nc.dram_tensor("wkT1", [C, D], f32r, kind="ExternalInput")
    wvT1_d = nc.dram_tensor("wvT1", [C, C], f32r, kind="ExternalInput")
    wqT2_d = nc.dram_tensor("wqT2", [C, D], f32r, kind="ExternalInput")
    wkT2_d = nc.dram_tensor("wkT2", [C, D], f32r, kind="ExternalInput")
    wvT2_d = nc.dram_tensor("wvT2", [C, C], f32r, kind="ExternalInput")
    # consts columns: 0 sa_gamma, 1 gamma, 2/3 sa_gamma*sa_bv halves,
    # 6 sa_bq, 7 sa_bk, 8 bq, 9 bk (cols 6-9 live on partitions 0..31)
    consts_d = nc.dram_tensor("consts", [P, 10], f32, kind="ExternalInput")
    out_d = nc.dram_tensor("outc", [C, R], f32, kind="ExternalOutput")

    groups = [[0, 1, 2, 3], [4, 5, 6, 7]]

    with tile.TileContext(nc) as tc:
        from contextlib import ExitStack

        ctx = ExitStack()
        with ctx:
            big = ctx.enter_context(tc.tile_pool(name="big", bufs=1))
            epool = ctx.enter_context(tc.tile_pool(name="epool", bufs=4))
            onpool = ctx.enter_context(tc.tile_pool(name="onpool", bufs=3))
            sqpool = ctx.enter_context(tc.tile_pool(name="sqpool", bufs=2))
            fbpool = ctx.enter_context(tc.tile_pool(name="fbpool", bufs=4))
            rcpool = ctx.enter_context(tc.tile_pool(name="rcpool", bufs=4))
            finpool = ctx.enter_context(tc.tile_pool(name="finpool", bufs=2))
            misc = ctx.enter_context(tc.tile_pool(name="misc", bufs=1))
            psA = ctx.enter_context(
                tc.tile_pool(name="psA", bufs=2, space="PSUM")
            )
            psS = ctx.enter_context(
                tc.tile_pool(name="psS", bufs=3, space="PSUM")
            )
            psO = ctx.enter_context(
                tc.tile_pool(name="psO", bufs=3, space="PSUM")
            )
            dram = ctx.enter_context(
                tc.tile_pool(name="dram", bufs=1, space="DRAM")
            )

            # ------------- persistent SBUF tiles -------------
            xc_sb = big.tile([P, 2, R], f32r, tag="xc", name="xc_sb")
            mask_sb = big.tile([P, N], f32, tag="mask", name="mask_sb")
            maskc_sb = big.tile([P, R], f32, tag="maskc", name="maskc_sb")
            xp_sb = big.tile([P, 2, R], f32, tag="xp", name="xp_sb")
            ff_sb = big.tile([P, 2, R], f32r, tag="ff", name="ff_sb")
            wqT1_sb = big.tile([P, 2, D], f32r, tag="wqT1", name="wqT1_sb")
            wkT1_sb = big.tile([P, 2, D], f32r, tag="wkT1", name="wkT1_sb")
            wvT1_sb = big.tile([P, 2, C], f32r, tag="wvT1", name="wvT1_sb")
            wqT2_sb = big.tile([P, 2, D], f32r, tag="wqT2", name="wqT2_sb")
            wkT2_sb = big.tile([P, 2, D], f32r, tag="wkT2", name="wkT2_sb")
            wvT2_sb = big.tile([P, 2, C], f32r, tag="wvT2", name="wvT2_sb")
            consts_sb = big.tile([P, 10], f32, tag="consts", name="consts_sb")
            # ones column (f32r) for the denominator matmul; ones row (f32)
            # for the K=1 reciprocal-replication matmul
            onesc_sb = big.tile([P, 1], bf16, tag="onesc", name="onesc_sb")
            onesr_sb = big.tile([1, P], f32r, tag="onesr", name="onesr_sb")
            stats_sb = misc.tile([P, 8], f32, tag="stats", name="stats_sb")

            # input DMAs
            nc.sync.dma_start(out=consts_sb[:], in_=consts_d[:])
            for k in range(2):
                cs = slice(k * P, (k + 1) * P)
                nc.sync.dma_start(out=wqT1_sb[:, k, :], in_=wqT1_d[cs, :])
                nc.sync.dma_start(out=wkT1_sb[:, k, :], in_=wkT1_d[cs, :])
                nc.sync.dma_start(out=wvT1_sb[:, k, :], in_=wvT1_d[cs, :])
                nc.sync.dma_start(out=wqT2_sb[:, k, :], in_=wqT2_d[cs, :])
                nc.sync.dma_start(out=wkT2_sb[:, k, :], in_=wkT2_d[cs, :])
                nc.sync.dma_start(out=wvT2_sb[:, k, :], in_=wvT2_d[cs, :])
                nc.sync.dma_start(out=xc_sb[:, k, :], in_=xc_d[cs, :])
            nc.sync.dma_start(
                out=mask_sb[:], in_=mrow_d[0, :].partition_broadcast(P)
            )
            nc.sync.dma_start(
                out=maskc_sb[:], in_=mcrow_d[0, :].partition_broadcast(P)
            )
            nc.vector.memset(onesc_sb[:], 1.0)
            nc.vector.memset(onesr_sb[:].bitcast(f32), 1.0)

            # x_full: chunked DMA so layer-1 convs can start early.
            xf_sb = big.tile([P, 2, N], f32r, tag="xbig", name="xf_sb")
            for k in range(2):
                for jc in range(4):
                    js = slice(jc * (N // 4), (jc + 1) * (N // 4))
                    nc.sync.dma_start(
                        out=xf_sb[:, k, js],
                        in_=xf_d[k * P : (k + 1) * P, js],
                    )

            def conv_qk(wT_sb, bias_col, src_of, width, out_sb):
                """out (D x width) = wT.T @ src + bias.  src_of(k, js) gives
                the (128 x 512) input-channel tile."""
                for jc in range(width // IC):
                    js = slice(jc * IC, (jc + 1) * IC)
                    ps = psA.tile([D, IC], f32, tag="a", name="qk_ps")
                    nc.tensor.matmul(
                        ps[:], wT_sb[:, 0, :], src_of(0, js),
                        start=True, stop=False,
                    )
                    nc.tensor.matmul(
                        ps[:], wT_sb[:, 1, :], src_of(1, js),
                        start=False, stop=True,
                    )
                    nc.vector.tensor_scalar_add(
                        out_sb[:, js], ps[:],
                        consts_sb[0:D, bias_col : bias_col + 1],
                    )

            def conv_vT(wvT_sb, src_of, v_sb, t):
                """v_sb[:, t, :] = (src^T @ wvT) for key tile t (j on
                partitions, channels free)."""
                ts_ = slice(t * P, (t + 1) * P)
                ps = psA.tile([P, C], f32, tag="a", name="v_ps")
                nc.tensor.matmul(
                    ps[:], src_of(0, ts_), wvT_sb[:, 0, :],
                    start=True, stop=False,
                )
                nc.tensor.matmul(
                    ps[:], src_of(1, ts_), wvT_sb[:, 1, :],
                    start=False, stop=True,
                )
                nc.vector.tensor_copy(v_sb[:, t, :], ps[:])

            def attention(q_sb, k_sb, v_sb, epilogue):
                """Row-chunk attention.  Per i-chunk: S^T = K-tile^T Q
                (j on partitions), E = exp(S^T), then O(c,i) accumulates
                with V^T slices stationary and E moving; the denominator
                row comes from an M=1 ones matmul.  The S/exp stage is
                emitted two key-tiles ahead of AV/den so the in-order PE
                queue never stalls on the ACT exp.  epilogue(ich, accs,
                rrep) gets natural-layout unnormalized O accumulators and
                the partition-replicated reciprocal denominator (SBUF)."""
                LOOKAHEAD = 2
                for ich in range(R // IC):
                    is_ = slice(ich * IC, (ich + 1) * IC)
                    accs = [
                        psO.tile([P, IC], f32, tag="o", name="acc")
                        for _ in range(2)
                    ]
                    den = psA.tile([1, IC], f32, tag="a", name="den")
                    es = {}

                    def s_exp(t):
                        sps = psS.tile([P, IC], f32, tag="s", name="s_ps")
                        nc.tensor.matmul(
                            sps[:],
                            k_sb[:, t * P : (t + 1) * P],
                            q_sb[:, is_],
                            start=True, stop=True,
                        )
                        e_sb = epool.tile([P, IC], bf16, tag="e", name="e_sb")
                        nc.scalar.activation(e_sb[:], sps[:], AF.Exp)
                        es[t] = e_sb

                    for t in range(LOOKAHEAD):
                        s_exp(t)
                    for t in range(NT):
                        if t + LOOKAHEAD < NT:
                            s_exp(t + LOOKAHEAD)
                        e_sb = es.pop(t)
                        for ct in range(2):
                            nc.tensor.matmul(
                                accs[ct][:],
                                v_sb[:, t, ct * P : (ct + 1) * P],
                                e_sb[:],
                                start=(t == 0), stop=(t == NT - 1),
                            )
                        nc.tensor.matmul(
                            den[:], onesc_sb[:], e_sb[:],
                            start=(t == 0), stop=(t == NT - 1),
                        )
                    # reciprocal of the denominator row, replicated to all
                    # partitions via a K=1 ones matmul (f32r: single-pass)
                    rrow = rcpool.tile([1, IC], f32, tag="rc", name="rrow")
                    nc.vector.reciprocal(rrow[:], den[:])
                    rrow_r = rcpool.tile([1, IC], f32r, tag="rcr", name="rrow_r")
                    nc.vector.tensor_copy(rrow_r[:], rrow[:])
                    rrep_ps = psA.tile([P, IC], f32, tag="a", name="rrep_ps")
                    nc.tensor.matmul(
                        rrep_ps[:], onesr_sb[:], rrow_r[:],
                        start=True, stop=True,
                    )
                    rrep = onpool.tile([P, IC], f32, tag="rr", name="rrep")
                    nc.scalar.copy(rrep[:], rrep_ps[:])
                    epilogue(ich, accs, rrep)

            # ================= Layer 1: self-attention =================
            q1_sb = big.tile([D, R], f32r, tag="q", name="q1_sb")
            k1_sb = big.tile([D, N], f32r, tag="k", name="k1_sb")
            v1_sb = big.tile([P, NT, C], bf16, tag="v", name="v1_sb")

            conv_qk(wqT1_sb, 6, lambda k, js: xc_sb[:, k, js], R, q1_sb)
            conv_qk(wkT1_sb, 7, lambda k, js: xf_sb[:, k, js], N, k1_sb)
            for t in range(NT):
                conv_vT(wvT1_sb, lambda k, ts_: xf_sb[:, k, ts_], v1_sb, t)

            def epilogue1(ich, accs, rrep):
                io = slice(ich * IC, (ich + 1) * IC)
                for ct in range(2):
                    # x' = sa_gamma * (O/den) + sa_gamma*bv + x
                    nc.vector.tensor_mul(
                        xp_sb[:, ct, io], accs[ct][:], rrep[:]
                    )
                    nc.vector.tensor_scalar(
                        xp_sb[:, ct, io], xp_sb[:, ct, io],
                        consts_sb[:, 0:1], consts_sb[:, 2 + ct : 3 + ct],
                        op0=OP.mult, op1=OP.add,
                    )
                    nc.vector.tensor_add(
                        xp_sb[:, ct, io],
                        xp_sb[:, ct, io],
                        xc_sb[:, ct, io].bitcast(f32),
                    )

            attention(q1_sb, k1_sb, v1_sb, epilogue1)

            # ====== AllGather x' within each batch group (2 phases) ======
            # Phase h gathers x' columns [h*512, (h+1)*512) of every rank;
            # phase 0 overlaps the second layer-1 attention i-chunk.
            xpf_sb = big.tile([P, 2, N], f32, tag="xbig", name="xpf_sb")
            for h in range(2):
                hs = slice(h * IC, (h + 1) * IC)
                ag_in = dram.tile(
                    [C, IC], f32, tag=f"ag_in{h}", name=f"ag_in{h}"
                )
                ag_out = dram.tile(
                    [RSH, C, IC], f32, tag=f"ag_out{h}", name=f"ag_out{h}"
                )
                for ct in range(2):
                    nc.sync.dma_start(
                        out=ag_in[ct * P : (ct + 1) * P, :],
                        in_=xp_sb[:, ct, hs],
                    )
                nc.gpsimd.collective_compute(
                    "AllGather",
                    OP.bypass,
                    replica_groups=groups,
                    ins=[ag_in[:].opt()],
                    outs=[ag_out[:].opt()],
                )
                for ct in range(2):
                    for r in range(RSH):
                        nc.sync.dma_start(
                            out=xpf_sb[
                                :, ct, r * R + h * IC : r * R + (h + 1) * IC
                            ],
                            in_=ag_out[r, ct * P : (ct + 1) * P, :],
                        )

            # ============== Layer 2: masked cross-attention ==============
            # feature_f chunk + its per-channel stats (cols 0-3 of stats_sb)
            ffsq = misc.tile([P, R], f32, tag="ffsq", name="ffsq")
            for ct in range(2):
                nc.vector.tensor_mul(
                    ff_sb[:, ct, :], maskc_sb[:], xp_sb[:, ct, :]
                )
                nc.vector.tensor_reduce(
                    stats_sb[:, ct : ct + 1], ff_sb[:, ct, :].bitcast(f32),
                    axis=AX.X, op=OP.add,
                )
                nc.vector.tensor_mul(
                    ffsq[:],
                    ff_sb[:, ct, :].bitcast(f32),
                    ff_sb[:, ct, :].bitcast(f32),
                )
                nc.vector.tensor_reduce(
                    stats_sb[:, 2 + ct : 3 + ct], ffsq[:],
                    axis=AX.X, op=OP.add,
                )

            q2_sb = big.tile([D, R], f32r, tag="q", name="q2_sb")
            conv_qk(wqT2_sb, 8, lambda k, js: ff_sb[:, k, js], R, q2_sb)

            # feature_b tiles on the fly -> K2 and V2^T convs.  Chunk order
            # interleaves gather phases: even chunks only need AG phase 0.
            k2_sb = big.tile([D, N], f32r, tag="k", name="k2_sb")
            v2_sb = big.tile([P, NT, C], bf16, tag="v", name="v2_sb")
            for jc in (0, 2, 4, 6, 1, 3, 5, 7):
                js = slice(jc * IC, (jc + 1) * IC)
                fb0 = fbpool.tile([P, IC], f32r, tag="fb", name="fb0")
                fb1 = fbpool.tile([P, IC], f32r, tag="fb", name="fb1")
                # fb = (1-mask)*x' = x' - mask*x'
                nc.vector.tensor_mul(
                    fb0[:], mask_sb[:, js], xpf_sb[:, 0, js]
                )
                nc.vector.tensor_sub(
                    fb0[:], xpf_sb[:, 0, js], fb0[:].bitcast(f32)
                )
                nc.vector.tensor_mul(
                    fb1[:], mask_sb[:, js], xpf_sb[:, 1, js]
                )
                nc.vector.tensor_sub(
                    fb1[:], xpf_sb[:, 1, js], fb1[:].bitcast(f32)
                )
                ps = psA.tile([D, IC], f32, tag="a", name="k2_ps")
                nc.tensor.matmul(
                    ps[:], wkT2_sb[:, 0, :], fb0[:], start=True, stop=False
                )
                nc.tensor.matmul(
                    ps[:], wkT2_sb[:, 1, :], fb1[:], start=False, stop=True
                )
                nc.vector.tensor_scalar_add(
                    k2_sb[:, js], ps[:], consts_sb[0:D, 9:10]
                )
                for tsub in range(IC // P):
                    t = jc * (IC // P) + tsub
                    ts_ = slice(tsub * P, (tsub + 1) * P)
                    psv = psA.tile([P, C], f32, tag="a", name="v2_ps")
                    nc.tensor.matmul(
                        psv[:], fb0[:, ts_], wvT2_sb[:, 0, :],
                        start=True, stop=False,
                    )
                    nc.tensor.matmul(
                        psv[:], fb1[:, ts_], wvT2_sb[:, 1, :],
                        start=False, stop=True,
                    )
                    nc.vector.tensor_copy(v2_sb[:, t, :], psv[:])

            def epilogue2(ich, accs, rrep):
                # normalized sw_bg chunk in natural layout; accumulate
                # per-channel sum/sumsq into stats_sb cols 4-7 via VE
                for ct in range(2):
                    onb = onpool.tile([P, IC], f32, tag="on", name="on2")
                    nc.vector.tensor_mul(onb[:], accs[ct][:], rrep[:])
                    s1 = rcpool.tile([P, 1], f32, tag="s1", name="s1")
                    nc.vector.tensor_reduce(
                        s1[:], onb[:], axis=AX.X, op=OP.add
                    )
                    sqb = sqpool.tile([P, IC], f32, tag="sq", name="sq2")
                    nc.vector.tensor_mul(sqb[:], onb[:], onb[:])
                    s2 = rcpool.tile([P, 1], f32, tag="s2", name="s2")
                    nc.vector.tensor_reduce(
                        s2[:], sqb[:], axis=AX.X, op=OP.add
                    )
                    if ich == 0:
                        nc.vector.tensor_copy(
                            stats_sb[:, 4 + ct : 5 + ct], s1[:]
                        )
                        nc.vector.tensor_copy(
                            stats_sb[:, 6 + ct : 7 + ct], s2[:]
                        )
                    else:
                        nc.vector.tensor_add(
                            stats_sb[:, 4 + ct : 5 + ct],
                            stats_sb[:, 4 + ct : 5 + ct], s1[:],
                        )
                        nc.vector.tensor_add(
                            stats_sb[:, 6 + ct : 7 + ct],
                            stats_sb[:, 6 + ct : 7 + ct], s2[:],
                        )

            attention(q2_sb, k2_sb, v2_sb, epilogue2)

            # ================== stats AllReduce + FMM ==================
            ar_in = dram.tile([P, 8], f32, tag="ar_in", name="ar_in")
            ar_out = dram.tile([P, 8], f32, tag="ar_out", name="ar_out")
            nc.sync.dma_start(out=ar_in[:], in_=stats_sb[:])
            nc.gpsimd.collective_compute(
                "AllReduce",
                OP.add,
                replica_groups=groups,
                ins=[ar_in[:].opt()],
                outs=[ar_out[:].opt()],
            )
            rst = misc.tile([P, 8], f32, tag="rst", name="rst")
            nc.sync.dma_start(out=rst[:], in_=ar_out[:])

            # var = (S2 - S1^2/N)/(N-1) + EPS (both channel-halves at once)
            varf = misc.tile([P, 2], f32, tag="varf", name="varf")
            varg = misc.tile([P, 2], f32, tag="varg", name="varg")
            ratio = misc.tile([P, 2], f32, tag="ratio", name="ratio")
            for var, s1s, s2s in ((varf, 0, 2), (varg, 4, 6)):
                nc.vector.tensor_mul(
                    var[:], rst[:, s1s : s1s + 2], rst[:, s1s : s1s + 2]
                )
                nc.vector.tensor_scalar(
                    var[:], var[:], -1.0 / N, None, op0=OP.mult
                )
                nc.vector.tensor_add(var[:], var[:], rst[:, s2s : s2s + 2])
                nc.vector.tensor_scalar(
                    var[:], var[:], 1.0 / (N - 1), EPS, op0=OP.mult, op1=OP.add
                )
            nc.vector.reciprocal(varf[:], varf[:])
            nc.vector.tensor_mul(varg[:], varg[:], varf[:])
            nc.scalar.activation(ratio[:], varg[:], AF.Sqrt)
            # fold in gamma
            nc.vector.tensor_scalar_mul(ratio[:], ratio[:], consts_sb[:, 1:2])

            # out = x' + (gamma * std_bg/std_f) * ff
            for ct in range(2):
                fin = finpool.tile([P, R], f32, tag="fin", name="fin")
                nc.vector.tensor_scalar_mul(
                    fin[:], ff_sb[:, ct, :].bitcast(f32),
                    ratio[:, ct : ct + 1],
                )
                nc.vector.tensor_add(fin[:], fin[:], xp_sb[:, ct, :])
                nc.sync.dma_start(
                    out=out_d[ct * P : (ct + 1) * P, :], in_=fin[:]
                )

    nc.compile()
    return nc


def _prep_inputs(x, mask, sa_wq, sa_bq, sa_wk, sa_bk, sa_wv, sa_bv, sa_gamma,
                 wq, bq, wk, bk, wv, bv, gamma):
    """Build the per-core input maps (host-side sharding + weight layout)."""
    x = np.ascontiguousarray(x, dtype=F32)
    mask = np.ascontiguousarray(mask, dtype=F32)

    wqT1 = np.ascontiguousarray(sa_wq.T, dtype=F32)
    wkT1 = np.ascontiguousarray(sa_wk.T, dtype=F32)
    wvT1 = np.ascontiguousarray(sa_wv.T, dtype=F32)
    wqT2 = np.ascontiguousarray(wq.T, dtype=F32)
    wkT2 = np.ascontiguousarray(wk.T, dtype=F32)
    wvT2 = np.ascontiguousarray(wv.T, dtype=F32)

    consts = np.zeros((P, 10), dtype=F32)
    consts[:, 0] = sa_gamma[0]
    consts[:, 1] = gamma[0]
    sgb = (sa_gamma[0] * sa_bv).astype(F32)
    consts[:, 2] = sgb[0:P]
    consts[:, 3] = sgb[P:C]
    consts[0:D, 6] = sa_bq
    consts[0:D, 7] = sa_bk
    consts[0:D, 8] = bq
    consts[0:D, 9] = bk

    in_maps = []
    for g in range(NCORES):
        b, r = g // RSH, g % RSH
        xb = np.ascontiguousarray(x[b].reshape(C, N))
        mb = np.ascontiguousarray(mask[b].reshape(1, N))
        in_maps.append({
            "xf": xb,
            "xc": np.ascontiguousarray(xb[:, r * R : (r + 1) * R]),
            "mrow": mb,
            "mcrow": np.ascontiguousarray(mb[:, r * R : (r + 1) * R]),
            "wqT1": wqT1, "wkT1": wkT1, "wvT1": wvT1,
            "wqT2": wqT2, "wkT2": wkT2, "wvT2": wvT2,
            "consts": consts,
        })
    return in_maps


def kernel(**inputs):
    from concourse import bass_utils

    if "nc" not in _CACHE:
        _CACHE["nc"] = _build_bass()
    nc = _CACHE["nc"]

    in_maps = _prep_inputs(**inputs)
    res = bass_utils.run_bass_kernel_spmd(
        nc, in_maps, core_ids=list(range(NCORES))
    )
    _CACHE["last_results"] = res

    out = np.empty((B, C, N), dtype=F32)
    for g in range(NCORES):
        b, r = g // RSH, g % RSH
        out[b, :, r * R : (r + 1) * R] = res.results[g]["outc"]
    return out.reshape(B, C, HH, WW)


# revision 13
# speedup vs baseline: 1.1432x; 1.0793x over previous
"""Trainium2 Bass/Tile kernel for nn_FB_FMM (sparse_attention).

Computation (per batch element b, with N = H*W = 4096 tokens, C=256, D=32):
  1. Self-attention:  sa_out = attn(conv(x,sa_wq), conv(x,sa_wk), conv(x,sa_wv))
     x' = sa_gamma * sa_out + x
  2. Masked cross-attention (FB_FMM):
     ff = mask * x'; fb = (1-mask) * x'
     sw_bg = attn(conv(ff,wq), conv(fb,wk), conv(fb,wv))
     out = x' + gamma * ff * (std(sw_bg)/std(ff))    [per-channel std, ddof=1]

Sharding: 8 cores = 2 batch groups x 4-way query-row sharding (1024 rows each).
Each core computes its row-chunk of both attention layers; K/V sides are
computed redundantly per core (cheap: D=32 / one C x C conv). Cross-core
communication inside the kernel:
  - AllGather of x' chunks within each 4-core batch group (layer-2 K/V need
    the full x'), split into two 512-row phases so the first overlaps the
    second half of the layer-1 attention loop.
  - AllReduce of per-channel [sum, sumsq] stats for the FMM std ratio.

Layouts: feature maps are channel-major (C on partitions). Scores are computed
transposed (S^T: keys j on partitions, queries i free; logits are small so exp
needs no max-subtraction pass). The AV matmul keeps V^T slices stationary
(weight reuse) with E^T moving, producing O in natural (c x i) layout; the
softmax denominator comes from one extra M=1 ones-matmul per tile, and the
reciprocal row is broadcast across partitions with a K=1 ones matmul. All
heavy matmuls run in float32r (1 cycle/row vs 4 for fp32; ~1e-4 relative
rounding). V-conv biases are folded out mathematically (sum_j A[i,j] = 1
makes the layer-1 V bias a constant shift folded into the residual; variance
is shift-invariant so the layer-2 V bias drops out of the FMM std).
"""

import numpy as np

P = 128
B, C, HH, WW = 2, 256, 64, 64
N = HH * WW            # 4096 tokens
D = 32                 # q/k channels
NCORES = 8
RSH = 4                # row shards per batch group
R = N // RSH           # 1024 query rows per core
NT = N // P            # 32 key tiles
IC = 512               # query i-chunk (one PSUM bank of fp32)
EPS = 1e-5
F32 = np.float32

_CACHE = {}


def _build_bass():
    """Build the Bass/Tile program (single SPMD NEFF for all 8 cores)."""
    import concourse.bass as bass
    from concourse import bacc, mybir, tile

    f32 = mybir.dt.float32
    f32r = mybir.dt.float32r
    bf16 = mybir.dt.bfloat16
    AX = mybir.AxisListType
    OP = mybir.AluOpType
    AF = mybir.ActivationFunctionType

    nc = bacc.Bacc(
        "TRN2", target_bir_lowering=False, debug=False, num_devices=NCORES
    )

    # ---------------- I/O ----------------
    xf_d = nc.dram_tensor("xf", [C, N], f32r, kind="ExternalInput")
    xc_d = nc.dram_tensor("xc", [C, R], f32r, kind="ExternalInput")
    mrow_d = nc.dram_tensor("mrow", [1, N], f32, kind="ExternalInput")
    mcrow_d = nc.dram_tensor("mcrow", [1, R], f32, kind="ExternalInput")
    wqT1_d = nc.dram_tensor("wqT1", [C, D], f32r, kind="ExternalInput")
    wkT1_d = nc.dram_tensor("wkT1", [C, D], f32r, kind="ExternalInput")
    wvT1_d = nc.dram_tensor("wvT1", [C, C], f32r, kind="ExternalInput")
    wqT2_d = nc.dram_tensor("wqT2", [C, D], f32r, kind="ExternalInput")
    wkT2_d = nc.dram_tensor("wkT2", [C, D], f32r, kind="ExternalInput")
    wvT2_d = nc.dram_tensor("wvT2", [C, C], f32r, kind="ExternalInput")
    # consts columns: 0 sa_gamma, 1 gamma, 2/3 sa_gamma*sa_bv halves,
    # 6 sa_bq, 7 sa_bk, 8 bq, 9 bk (cols 6-9 live on partitions 0..31)
    consts_d = nc.dram_tensor("consts", [P, 10], f32, kind="ExternalInput")
    out_d = nc.dram_tensor("outc", [C, R], f32, kind="ExternalOutput")

    groups = [[0, 1, 2, 3], [4, 5, 6, 7]]

    with tile.TileContext(nc) as tc:
        from contextlib import ExitStack

        ctx = ExitStack()
        with ctx:
            big = ctx.enter_context(tc.tile_pool(name="big", bufs=1))
            epool = ctx.enter_context(tc.tile_pool(name="epool", bufs=4))
            onpool = ctx.enter_context(tc.tile_pool(name="onpool", bufs=3))
            sqpool = ctx.enter_context(tc.tile_pool(name="sqpool", bufs=2))
            fbpool = ctx.enter_context(tc.tile_pool(name="fbpool", bufs=4))
            rcpool = ctx.enter_context(tc.tile_pool(name="rcpool", bufs=4))
            finpool = ctx.enter_context(tc.tile_pool(name="finpool", bufs=2))
            misc = ctx.enter_context(tc.tile_pool(name="misc", bufs=1))
            psA = ctx.enter_context(
                tc.tile_pool(name="psA", bufs=2, space="PSUM")
            )
            psS = ctx.enter_context(
                tc.tile_pool(name="psS", bufs=3, space="PSUM")
            )
            psO = ctx.enter_context(
                tc.tile_pool(name="psO", bufs=3, space="PSUM")
            )
            dram = ctx.enter_context(
                tc.tile_pool(name="dram", bufs=1, space="DRAM")
            )

            # ------------- persistent SBUF tiles -------------
            xc_sb = big.tile([P, 2, R], f32r, tag="xc", name="xc_sb")
            mask_sb = big.tile([P, N], f32, tag="mask", name="mask_sb")
            maskc_sb = big.tile([P, R], f32, tag="maskc", name="maskc_sb")
            xp_sb = big.tile([P, 2, R], f32, tag="xp", name="xp_sb")
            ff_sb = big.tile([P, 2, R], f32r, tag="ff", name="ff_sb")
            wqT1_sb = big.tile([P, 2, D], f32r, tag="wqT1", name="wqT1_sb")
            wkT1_sb = big.tile([P, 2, D], f32r, tag="wkT1", name="wkT1_sb")
            wvT1_sb = big.tile([P, 2, C], f32r, tag="wvT1", name="wvT1_sb")
            wqT2_sb = big.tile([P, 2, D], f32r, tag="wqT2", name="wqT2_sb")
            wkT2_sb = big.tile([P, 2, D], f32r, tag="wkT2", name="wkT2_sb")
            wvT2_sb = big.tile([P, 2, C], f32r, tag="wvT2", name="wvT2_sb")
            consts_sb = big.tile([P, 10], f32, tag="consts", name="consts_sb")
            # ones column (f32r) for the denominator matmul; ones row (f32)
            # for the K=1 reciprocal-replication matmul
            onesc_sb = big.tile([P, 1], bf16, tag="onesc", name="onesc_sb")
            onesr_sb = big.tile([1, P], f32r, tag="onesr", name="onesr_sb")
            stats_sb = misc.tile([P, 8], f32, tag="stats", name="stats_sb")

            # input DMAs (x first: the conv pipeline is gated on it)
            xf_sb = big.tile([P, 2, N], f32r, tag="xbig", name="xf_sb")
            for k in range(2):
                for jc in range(8):
                    js = slice(jc * (N // 8), (jc + 1) * (N // 8))
                    nc.sync.dma_start(
                        out=xf_sb[:, k, js],
                        in_=xf_d[k * P : (k + 1) * P, js],
                    )
            nc.sync.dma_start(out=consts_sb[:], in_=consts_d[:])
            for k in range(2):
                cs = slice(k * P, (k + 1) * P)
                nc.sync.dma_start(out=wqT1_sb[:, k, :], in_=wqT1_d[cs, :])
                nc.sync.dma_start(out=wkT1_sb[:, k, :], in_=wkT1_d[cs, :])
                nc.sync.dma_start(out=wvT1_sb[:, k, :], in_=wvT1_d[cs, :])
                nc.sync.dma_start(out=wqT2_sb[:, k, :], in_=wqT2_d[cs, :])
                nc.sync.dma_start(out=wkT2_sb[:, k, :], in_=wkT2_d[cs, :])
                nc.sync.dma_start(out=wvT2_sb[:, k, :], in_=wvT2_d[cs, :])
                nc.sync.dma_start(out=xc_sb[:, k, :], in_=xc_d[cs, :])
            nc.sync.dma_start(
                out=mask_sb[:], in_=mrow_d[0, :].partition_broadcast(P)
            )
            nc.sync.dma_start(
                out=maskc_sb[:], in_=mcrow_d[0, :].partition_broadcast(P)
            )
            nc.vector.memset(onesc_sb[:], 1.0)
            nc.vector.memset(onesr_sb[:].bitcast(f32), 1.0)

            def conv_qk(wT_sb, bias_col, src_of, width, out_sb):
                """out (D x width) = wT.T @ src + bias.  src_of(k, js) gives
                the (128 x 512) input-channel tile."""
                for jc in range(width // IC):
                    js = slice(jc * IC, (jc + 1) * IC)
                    ps = psA.tile([D, IC], f32, tag="a", name="qk_ps")
                    nc.tensor.matmul(
                        ps[:], wT_sb[:, 0, :], src_of(0, js),
                        start=True, stop=False,
                    )
                    nc.tensor.matmul(
                        ps[:], wT_sb[:, 1, :], src_of(1, js),
                        start=False, stop=True,
                    )
                    nc.vector.tensor_scalar_add(
                        out_sb[:, js], ps[:],
                        consts_sb[0:D, bias_col : bias_col + 1],
                    )

            def conv_vT(wvT_sb, src_of, v_sb, t):
                """v_sb[:, t, :] = (src^T @ wvT) for key tile t (j on
                partitions, channels free)."""
                ts_ = slice(t * P, (t + 1) * P)
                ps = psA.tile([P, C], f32, tag="a", name="v_ps")
                nc.tensor.matmul(
                    ps[:], src_of(0, ts_), wvT_sb[:, 0, :],
                    start=True, stop=False,
                )
                nc.tensor.matmul(
                    ps[:], src_of(1, ts_), wvT_sb[:, 1, :],
                    start=False, stop=True,
                )
                nc.vector.tensor_copy(v_sb[:, t, :], ps[:])

            def attention(q_sb, k_sb, v_sb, epilogue):
                """Row-chunk attention.  Per i-chunk: S^T = K-tile^T Q
                (j on partitions), E = exp(S^T), then O(c,i) accumulates
                with V^T slices stationary and E moving; the denominator
                row comes from an M=1 ones matmul.  The S/exp stage is
                emitted two key-tiles ahead of AV/den so the in-order PE
                queue never stalls on the ACT exp.  epilogue(ich, accs,
                rrep) gets natural-layout unnormalized O accumulators and
                the partition-replicated reciprocal denominator (SBUF)."""
                LOOKAHEAD = 2
                for ich in range(R // IC):
                    is_ = slice(ich * IC, (ich + 1) * IC)
                    accs = [
                        psO.tile([P, IC], f32, tag="o", name="acc")
                        for _ in range(2)
                    ]
                    den = psA.tile([1, IC], f32, tag="a", name="den")
                    es = {}

                    def s_exp(t):
                        sps = psS.tile([P, IC], f32, tag="s", name="s_ps")
                        nc.tensor.matmul(
                            sps[:],
                            k_sb[:, t * P : (t + 1) * P],
                            q_sb[:, is_],
                            start=True, stop=True,
                        )
                        e_sb = epool.tile([P, IC], bf16, tag="e", name="e_sb")
                        nc.scalar.activation(e_sb[:], sps[:], AF.Exp)
                        es[t] = e_sb

                    for t in range(LOOKAHEAD):
                        s_exp(t)
                    for t in range(NT):
                        if t + LOOKAHEAD < NT:
                            s_exp(t + LOOKAHEAD)
                        e_sb = es.pop(t)
                        for ct in range(2):
                            nc.tensor.matmul(
                                accs[ct][:],
                                v_sb[:, t, ct * P : (ct + 1) * P],
                                e_sb[:],
                                start=(t == 0), stop=(t == NT - 1),
                            )
                        nc.tensor.matmul(
                            den[:], onesc_sb[:], e_sb[:],
                            start=(t == 0), stop=(t == NT - 1),
                        )
                    # reciprocal of the denominator row, replicated to all
                    # partitions via a K=1 ones matmul (f32r: single-pass)
                    rrow = rcpool.tile([1, IC], f32, tag="rc", name="rrow")
                    nc.vector.reciprocal(rrow[:], den[:])
                    rrow_r = rcpool.tile([1, IC], f32r, tag="rcr", name="rrow_r")
                    nc.vector.tensor_copy(rrow_r[:], rrow[:])
                    rrep_ps = psA.tile([P, IC], f32, tag="a", name="rrep_ps")
                    nc.tensor.matmul(
                        rrep_ps[:], onesr_sb[:], rrow_r[:],
                        start=True, stop=True,
                    )
                    rrep = onpool.tile([P, IC], f32, tag="rr", name="rrep")
                    nc.scalar.copy(rrep[:], rrep_ps[:])
                    epilogue(ich, accs, rrep)

            # ================= Layer 1: self-attention =================
            q1_sb = big.tile([D, R], bf16, tag="q", name="q1_sb")
            k1_sb = big.tile([D, N], bf16, tag="k", name="k1_sb")
            v1_sb = big.tile([P, NT, C], bf16, tag="v", name="v1_sb")

            conv_qk(wqT1_sb, 6, lambda k, js: xc_sb[:, k, js], R, q1_sb)
            conv_qk(wkT1_sb, 7, lambda k, js: xf_sb[:, k, js], N, k1_sb)
            for t in range(NT):
                conv_vT(wvT1_sb, lambda k, ts_: xf_sb[:, k, ts_], v1_sb, t)

            def epilogue1(ich, accs, rrep):
                io = slice(ich * IC, (ich + 1) * IC)
                for ct in range(2):
                    # x' = sa_gamma * (O/den) + sa_gamma*bv + x
                    nc.vector.tensor_mul(
                        xp_sb[:, ct, io], accs[ct][:], rrep[:]
                    )
                    nc.vector.tensor_scalar(
                        xp_sb[:, ct, io], xp_sb[:, ct, io],
                        consts_sb[:, 0:1], consts_sb[:, 2 + ct : 3 + ct],
                        op0=OP.mult, op1=OP.add,
                    )
                    nc.vector.tensor_add(
                        xp_sb[:, ct, io],
                        xp_sb[:, ct, io],
                        xc_sb[:, ct, io].bitcast(f32),
                    )

            attention(q1_sb, k1_sb, v1_sb, epilogue1)

            # ====== AllGather x' within each batch group (2 phases) ======
            # Phase h gathers x' columns [h*512, (h+1)*512) of every rank;
            # phase 0 overlaps the second layer-1 attention i-chunk.
            xpf_sb = big.tile([P, 2, N], f32, tag="xbig", name="xpf_sb")
            for h in range(2):
                hs = slice(h * IC, (h + 1) * IC)
                ag_in = dram.tile(
                    [C, IC], f32, tag=f"ag_in{h}", name=f"ag_in{h}"
                )
                ag_out = dram.tile(
                    [RSH, C, IC], f32, tag=f"ag_out{h}", name=f"ag_out{h}"
                )
                for ct in range(2):
                    nc.sync.dma_start(
                        out=ag_in[ct * P : (ct + 1) * P, :],
                        in_=xp_sb[:, ct, hs],
                    )
                nc.gpsimd.collective_compute(
                    "AllGather",
                    OP.bypass,
                    replica_groups=groups,
                    ins=[ag_in[:].opt()],
                    outs=[ag_out[:].opt()],
                )
                for ct in range(2):
                    for r in range(RSH):
                        nc.sync.dma_start(
                            out=xpf_sb[
                                :, ct, r * R + h * IC : r * R + (h + 1) * IC
                            ],
                            in_=ag_out[r, ct * P : (ct + 1) * P, :],
                        )

            # ============== Layer 2: masked cross-attention ==============
            # feature_f chunk + its per-channel stats (cols 0-3 of stats_sb)
            ffsq = misc.tile([P, R], f32, tag="ffsq", name="ffsq")
            for ct in range(2):
                nc.vector.tensor_mul(
                    ff_sb[:, ct, :], maskc_sb[:], xp_sb[:, ct, :]
                )
                nc.vector.tensor_reduce(
                    stats_sb[:, ct : ct + 1], ff_sb[:, ct, :].bitcast(f32),
                    axis=AX.X, op=OP.add,
                )
                nc.vector.tensor_mul(
                    ffsq[:],
                    ff_sb[:, ct, :].bitcast(f32),
                    ff_sb[:, ct, :].bitcast(f32),
                )
                nc.vector.tensor_reduce(
                    stats_sb[:, 2 + ct : 3 + ct], ffsq[:],
                    axis=AX.X, op=OP.add,
                )

            q2_sb = big.tile([D, R], bf16, tag="q", name="q2_sb")
            conv_qk(wqT2_sb, 8, lambda k, js: ff_sb[:, k, js], R, q2_sb)

            # feature_b tiles on the fly -> K2 and V2^T convs.  Chunk order
            # interleaves gather phases: even chunks only need AG phase 0.
            k2_sb = big.tile([D, N], bf16, tag="k", name="k2_sb")
            v2_sb = big.tile([P, NT, C], bf16, tag="v", name="v2_sb")
            for jc in (0, 2, 4, 6, 1, 3, 5, 7):
                js = slice(jc * IC, (jc + 1) * IC)
                fb0 = fbpool.tile([P, IC], f32r, tag="fb", name="fb0")
                fb1 = fbpool.tile([P, IC], f32r, tag="fb", name="fb1")
                # fb = (1-mask)*x' = x' - mask*x'
                nc.vector.tensor_mul(
                    fb0[:], mask_sb[:, js], xpf_sb[:, 0, js]
                )
                nc.vector.tensor_sub(
                    fb0[:], xpf_sb[:, 0, js], fb0[:].bitcast(f32)
                )
                nc.vector.tensor_mul(
                    fb1[:], mask_sb[:, js], xpf_sb[:, 1, js]
                )
                nc.vector.tensor_sub(
                    fb1[:], xpf_sb[:, 1, js], fb1[:].bitcast(f32)
                )
                ps = psA.tile([D, IC], f32, tag="a", name="k2_ps")
                nc.tensor.matmul(
                    ps[:], wkT2_sb[:, 0, :], fb0[:], start=True, stop=False
                )
                nc.tensor.matmul(
                    ps[:], wkT2_sb[:, 1, :], fb1[:], start=False, stop=True
                )
                nc.vector.tensor_scalar_add(
                    k2_sb[:, js], ps[:], consts_sb[0:D, 9:10]
                )
                for tsub in range(IC // P):
                    t = jc * (IC // P) + tsub
                    ts_ = slice(tsub * P, (tsub + 1) * P)
                    psv = psA.tile([P, C], f32, tag="a", name="v2_ps")
                    nc.tensor.matmul(
                        psv[:], fb0[:, ts_], wvT2_sb[:, 0, :],
                        start=True, stop=False,
                    )
                    nc.tensor.matmul(
                        psv[:], fb1[:, ts_], wvT2_sb[:, 1, :],
                        start=False, stop=True,
                    )
                    nc.vector.tensor_copy(v2_sb[:, t, :], psv[:])

            def epilogue2(ich, accs, rrep):
                # normalized sw_bg chunk in natural layout; accumulate
                # per-channel sum/sumsq into stats_sb cols 4-7 via VE
                for ct in range(2):
                    onb = onpool.tile([P, IC], f32, tag="on", name="on2")
                    nc.vector.tensor_mul(onb[:], accs[ct][:], rrep[:])
                    s1 = rcpool.tile([P, 1], f32, tag="s1", name="s1")
                    nc.vector.tensor_reduce(
                        s1[:], onb[:], axis=AX.X, op=OP.add
                    )
                    sqb = sqpool.tile([P, IC], f32, tag="sq", name="sq2")
                    nc.vector.tensor_mul(sqb[:], onb[:], onb[:])
                    s2 = rcpool.tile([P, 1], f32, tag="s2", name="s2")
                    nc.vector.tensor_reduce(
                        s2[:], sqb[:], axis=AX.X, op=OP.add
                    )
                    if ich == 0:
                        nc.vector.tensor_copy(
                            stats_sb[:, 4 + ct : 5 + ct], s1[:]
                        )
                        nc.vector.tensor_copy(
                            stats_sb[:, 6 + ct : 7 + ct], s2[:]
                        )
                    else:
                        nc.vector.tensor_add(
                            stats_sb[:, 4 + ct : 5 + ct],
                            stats_sb[:, 4 + ct : 5 + ct], s1[:],
                        )
                        nc.vector.tensor_add(
                            stats_sb[:, 6 + ct : 7 + ct],
                            stats_sb[:, 6 + ct : 7 + ct], s2[:],
                        )

            attention(q2_sb, k2_sb, v2_sb, epilogue2)

            # ================== stats AllReduce + FMM ==================
            ar_in = dram.tile([P, 8], f32, tag="ar_in", name="ar_in")
            ar_out = dram.tile([P, 8], f32, tag="ar_out", name="ar_out")
            nc.sync.dma_start(out=ar_in[:], in_=stats_sb[:])
            nc.gpsimd.collective_compute(
                "AllReduce",
                OP.add,
                replica_groups=groups,
                ins=[ar_in[:].opt()],
                outs=[ar_out[:].opt()],
            )
            rst = misc.tile([P, 8], f32, tag="rst", name="rst")
            nc.sync.dma_start(out=rst[:], in_=ar_out[:])

            # var = (S2 - S1^2/N)/(N-1) + EPS (both channel-halves at once)
            varf = misc.tile([P, 2], f32, tag="varf", name="varf")
            varg = misc.tile([P, 2], f32, tag="varg", name="varg")
            ratio = misc.tile([P, 2], f32, tag="ratio", name="ratio")
            for var, s1s, s2s in ((varf, 0, 2), (varg, 4, 6)):
                nc.vector.tensor_mul(
                    var[:], rst[:, s1s : s1s + 2], rst[:, s1s : s1s + 2]
                )
                nc.vector.tensor_scalar(
                    var[:], var[:], -1.0 / N, None, op0=OP.mult
                )
                nc.vector.tensor_add(var[:], var[:], rst[:, s2s : s2s + 2])
                nc.vector.tensor_scalar(
                    var[:], var[:], 1.0 / (N - 1), EPS, op0=OP.mult, op1=OP.add
                )
            nc.vector.reciprocal(varf[:], varf[:])
            nc.vector.tensor_mul(varg[:], varg[:], varf[:])
            nc.scalar.activation(ratio[:], varg[:], AF.Sqrt)
            # fold in gamma
            nc.vector.tensor_scalar_mul(ratio[:], ratio[:], consts_sb[:, 1:2])

            # out = x' + (gamma * std_bg/std_f) * ff
            for ct in range(2):
                fin = finpool.tile([P, R], f32, tag="fin", name="fin")
                nc.vector.tensor_scalar_mul(
                    fin[:], ff_sb[:, ct, :].bitcast(f32),
                    ratio[:, ct : ct + 1],
                )
                nc.vector.tensor_add(fin[:], fin[:], xp_sb[:, ct, :])
                nc.sync.dma_start(
                    out=out_d[ct * P : (ct + 1) * P, :], in_=fin[:]
                )

    nc.compile()
    return nc


def _prep_inputs(x, mask, sa_wq, sa_bq, sa_wk, sa_bk, sa_wv, sa_bv, sa_gamma,
                 wq, bq, wk, bk, wv, bv, gamma):
    """Build the per-core input maps (host-side sharding + weight layout)."""
    x = np.ascontiguousarray(x, dtype=F32)
    mask = np.ascontiguousarray(mask, dtype=F32)

    wqT1 = np.ascontiguousarray(sa_wq.T, dtype=F32)
    wkT1 = np.ascontiguousarray(sa_wk.T, dtype=F32)
    wvT1 = np.ascontiguousarray(sa_wv.T, dtype=F32)
    wqT2 = np.ascontiguousarray(wq.T, dtype=F32)
    wkT2 = np.ascontiguousarray(wk.T, dtype=F32)
    wvT2 = np.ascontiguousarray(wv.T, dtype=F32)

    consts = np.zeros((P, 10), dtype=F32)
    consts[:, 0] = sa_gamma[0]
    consts[:, 1] = gamma[0]
    sgb = (sa_gamma[0] * sa_bv).astype(F32)
    consts[:, 2] = sgb[0:P]
    consts[:, 3] = sgb[P:C]
    consts[0:D, 6] = sa_bq
    consts[0:D, 7] = sa_bk
    consts[0:D, 8] = bq
    consts[0:D, 9] = bk

    in_maps = []
    for g in range(NCORES):
        b, r = g // RSH, g % RSH
        xb = np.ascontiguousarray(x[b].reshape(C, N))
        mb = np.ascontiguousarray(mask[b].reshape(1, N))
        in_maps.append({
            "xf": xb,
            "xc": np.ascontiguousarray(xb[:, r * R : (r + 1) * R]),
            "mrow": mb,
            "mcrow": np.ascontiguousarray(mb[:, r * R : (r + 1) * R]),
            "wqT1": wqT1, "wkT1": wkT1, "wvT1": wvT1,
            "wqT2": wqT2, "wkT2": wkT2, "wvT2": wvT2,
            "consts": consts,
        })
    return in_maps


def kernel(**inputs):
    from concourse import bass_utils

    if "nc" not in _CACHE:
        _CACHE["nc"] = _build_bass()
    nc = _CACHE["nc"]

    in_maps = _prep_inputs(**inputs)
    res = bass_utils.run_bass_kernel_spmd(
        nc, in_maps, core_ids=list(range(NCORES))
    )
    _CACHE["last_results"] = res

    out = np.empty((B, C, N), dtype=F32)
    for g in range(NCORES):
        b, r = g // RSH, g % RSH
        out[b, :, r * R : (r + 1) * R] = res.results[g]["outc"]
    return out.reshape(B, C, HH, WW)


# revision 14
# speedup vs baseline: 1.1957x; 1.0459x over previous
"""Trainium2 Bass/Tile kernel for nn_FB_FMM (sparse_attention).

Computation (per batch element b, with N = H*W = 4096 tokens, C=256, D=32):
  1. Self-attention:  sa_out = attn(conv(x,sa_wq), conv(x,sa_wk), conv(x,sa_wv))
     x' = sa_gamma * sa_out + x
  2. Masked cross-attention (FB_FMM):
     ff = mask * x'; fb = (1-mask) * x'
     sw_bg = attn(conv(ff,wq), conv(fb,wk), conv(fb,wv))
     out = x' + gamma * ff * (std(sw_bg)/std(ff))    [per-channel std, ddof=1]

Sharding: 8 cores = 2 batch groups x 4-way query-row sharding (1024 rows each).
Each core computes its row-chunk of both attention layers; K/V sides are
computed redundantly per core (cheap: D=32 / one C x C conv). Cross-core
communication inside the kernel:
  - AllGather of x' chunks within each 4-core batch group (layer-2 K/V need
    the full x'), split into two 512-row phases so the first overlaps the
    second half of the layer-1 attention loop.
  - AllReduce of per-channel [sum, sumsq] stats for the FMM std ratio.

Layouts: feature maps are channel-major (C on partitions). Scores are computed
transposed (S^T: keys j on partitions, queries i free; logits are small so exp
needs no max-subtraction pass). The AV matmul keeps V^T slices stationary
(weight reuse) with E^T moving, producing O in natural (c x i) layout; the
softmax denominator comes from one extra M=1 ones-matmul per tile, and the
reciprocal row is broadcast across partitions with a K=1 ones matmul. All
heavy matmuls run in float32r (1 cycle/row vs 4 for fp32; ~1e-4 relative
rounding). V-conv biases are folded out mathematically (sum_j A[i,j] = 1
makes the layer-1 V bias a constant shift folded into the residual; variance
is shift-invariant so the layer-2 V bias drops out of the FMM std).
"""

import numpy as np

P = 128
B, C, HH, WW = 2, 256, 64, 64
N = HH * WW            # 4096 tokens
D = 32                 # q/k channels
NCORES = 8
RSH = 4                # row shards per batch group
R = N // RSH           # 1024 query rows per core
NT = N // P            # 32 key tiles
IC = 512               # query i-chunk (one PSUM bank of fp32)
EPS = 1e-5
F32 = np.float32

_CACHE = {}


def _build_bass():
    """Build the Bass/Tile program (single SPMD NEFF for all 8 cores)."""
    import concourse.bass as bass
    from concourse import bacc, mybir, tile

    f32 = mybir.dt.float32
    f32r = mybir.dt.float32r
    bf16 = mybir.dt.bfloat16
    AX = mybir.AxisListType
    OP = mybir.AluOpType
    AF = mybir.ActivationFunctionType

    nc = bacc.Bacc(
        "TRN2", target_bir_lowering=False, debug=False, num_devices=NCORES
    )

    # ---------------- I/O ----------------
    xf_d = nc.dram_tensor("xf", [C, N], f32, kind="ExternalInput")
    xc_d = nc.dram_tensor("xc", [C, R], f32r, kind="ExternalInput")
    mrow_d = nc.dram_tensor("mrow", [1, N], f32, kind="ExternalInput")
    mcrow_d = nc.dram_tensor("mcrow", [1, R], f32, kind="ExternalInput")
    wqT1_d = nc.dram_tensor("wqT1", [C, D], f32r, kind="ExternalInput")
    wkT1_d = nc.dram_tensor("wkT1", [C, D], f32, kind="ExternalInput")
    wvT1_d = nc.dram_tensor("wvT1", [C, C], f32, kind="ExternalInput")
    wqT2_d = nc.dram_tensor("wqT2", [C, D], f32r, kind="ExternalInput")
    wkT2_d = nc.dram_tensor("wkT2", [C, D], f32, kind="ExternalInput")
    wvT2_d = nc.dram_tensor("wvT2", [C, C], f32, kind="ExternalInput")
    # consts columns: 0 sa_gamma, 1 gamma, 2/3 sa_gamma*sa_bv halves,
    # 6 sa_bq, 7 sa_bk, 8 bq, 9 bk (cols 6-9 live on partitions 0..31)
    consts_d = nc.dram_tensor("consts", [P, 10], f32, kind="ExternalInput")
    out_d = nc.dram_tensor("outc", [C, R], f32, kind="ExternalOutput")

    groups = [[0, 1, 2, 3], [4, 5, 6, 7]]

    with tile.TileContext(nc) as tc:
        from contextlib import ExitStack

        ctx = ExitStack()
        with ctx:
            big = ctx.enter_context(tc.tile_pool(name="big", bufs=1))
            epool = ctx.enter_context(tc.tile_pool(name="epool", bufs=4))
            onpool = ctx.enter_context(tc.tile_pool(name="onpool", bufs=3))
            sqpool = ctx.enter_context(tc.tile_pool(name="sqpool", bufs=2))
            fbpool = ctx.enter_context(tc.tile_pool(name="fbpool", bufs=4))
            rcpool = ctx.enter_context(tc.tile_pool(name="rcpool", bufs=4))
            finpool = ctx.enter_context(tc.tile_pool(name="finpool", bufs=2))
            misc = ctx.enter_context(tc.tile_pool(name="misc", bufs=1))
            psA = ctx.enter_context(
                tc.tile_pool(name="psA", bufs=2, space="PSUM")
            )
            psS = ctx.enter_context(
                tc.tile_pool(name="psS", bufs=3, space="PSUM")
            )
            psO = ctx.enter_context(
                tc.tile_pool(name="psO", bufs=3, space="PSUM")
            )
            dram = ctx.enter_context(
                tc.tile_pool(name="dram", bufs=1, space="DRAM")
            )

            # ------------- persistent SBUF tiles -------------
            xc_sb = big.tile([P, 2, R], f32r, tag="xc", name="xc_sb")
            mask_sb = big.tile([P, N], f32, tag="mask", name="mask_sb")
            maskc_sb = big.tile([P, R], f32, tag="maskc", name="maskc_sb")
            xp_sb = big.tile([P, 2, R], f32, tag="xp", name="xp_sb")
            ff_sb = big.tile([P, 2, R], f32r, tag="ff", name="ff_sb")
            wqT1_sb = big.tile([P, 2, D], f32r, tag="wqT1", name="wqT1_sb")
            wkT1_sb = big.tile([P, 2, D], bf16, tag="wkT1", name="wkT1_sb")
            wvT1_sb = big.tile([P, 2, C], bf16, tag="wvT1", name="wvT1_sb")
            wqT2_sb = big.tile([P, 2, D], f32r, tag="wqT2", name="wqT2_sb")
            wkT2_sb = big.tile([P, 2, D], bf16, tag="wkT2", name="wkT2_sb")
            wvT2_sb = big.tile([P, 2, C], bf16, tag="wvT2", name="wvT2_sb")
            consts_sb = big.tile([P, 10], f32, tag="consts", name="consts_sb")
            # ones column (f32r) for the denominator matmul; ones row (f32)
            # for the K=1 reciprocal-replication matmul
            onesc_sb = big.tile([P, 1], bf16, tag="onesc", name="onesc_sb")
            onesr_sb = big.tile([1, P], f32r, tag="onesr", name="onesr_sb")
            stats_sb = misc.tile([P, 8], f32, tag="stats", name="stats_sb")

            # input DMAs: small tensors first on the HWDGE queue; x-full
            # and the bf16 weight casts go through gpsimd (casting DMAs run
            # on a separate queue and overlap)
            xf_sb = big.tile([P, 2, N], bf16, tag="xbig", name="xf_sb")
            nc.sync.dma_start(out=consts_sb[:], in_=consts_d[:])
            for k in range(2):
                cs = slice(k * P, (k + 1) * P)
                nc.sync.dma_start(out=wqT1_sb[:, k, :], in_=wqT1_d[cs, :])
                nc.gpsimd.dma_start(out=wkT1_sb[:, k, :], in_=wkT1_d[cs, :])
                nc.gpsimd.dma_start(out=wvT1_sb[:, k, :], in_=wvT1_d[cs, :])
                nc.sync.dma_start(out=xc_sb[:, k, :], in_=xc_d[cs, :])
                nc.sync.dma_start(out=wqT2_sb[:, k, :], in_=wqT2_d[cs, :])
                nc.gpsimd.dma_start(out=wkT2_sb[:, k, :], in_=wkT2_d[cs, :])
                nc.gpsimd.dma_start(out=wvT2_sb[:, k, :], in_=wvT2_d[cs, :])
            nc.sync.dma_start(
                out=mask_sb[:], in_=mrow_d[0, :].partition_broadcast(P)
            )
            nc.sync.dma_start(
                out=maskc_sb[:], in_=mcrow_d[0, :].partition_broadcast(P)
            )
            for k in range(2):
                for jc in range(4):
                    js = slice(jc * (N // 4), (jc + 1) * (N // 4))
                    nc.gpsimd.dma_start(
                        out=xf_sb[:, k, js],
                        in_=xf_d[k * P : (k + 1) * P, js],
                    )
            nc.vector.memset(onesc_sb[:], 1.0)
            nc.vector.memset(onesr_sb[:].bitcast(f32), 1.0)

            def conv_qk(wT_sb, bias_col, src_of, width, out_sb):
                """out (D x width) = wT.T @ src + bias.  src_of(k, js) gives
                the (128 x 512) input-channel tile."""
                for jc in range(width // IC):
                    js = slice(jc * IC, (jc + 1) * IC)
                    ps = psA.tile([D, IC], f32, tag="a", name="qk_ps")
                    nc.tensor.matmul(
                        ps[:], wT_sb[:, 0, :], src_of(0, js),
                        start=True, stop=False,
                    )
                    nc.tensor.matmul(
                        ps[:], wT_sb[:, 1, :], src_of(1, js),
                        start=False, stop=True,
                    )
                    nc.vector.tensor_scalar_add(
                        out_sb[:, js], ps[:],
                        consts_sb[0:D, bias_col : bias_col + 1],
                    )

            def conv_vT(wvT_sb, src_of, v_sb, t):
                """v_sb[:, t, :] = (src^T @ wvT) for key tile t (j on
                partitions, channels free)."""
                ts_ = slice(t * P, (t + 1) * P)
                ps = psA.tile([P, C], f32, tag="a", name="v_ps")
                nc.tensor.matmul(
                    ps[:], src_of(0, ts_), wvT_sb[:, 0, :],
                    start=True, stop=False,
                )
                nc.tensor.matmul(
                    ps[:], src_of(1, ts_), wvT_sb[:, 1, :],
                    start=False, stop=True,
                )
                nc.vector.tensor_copy(v_sb[:, t, :], ps[:])

            def attention(q_sb, k_sb, v_sb, epilogue):
                """Row-chunk attention.  Per i-chunk: S^T = K-tile^T Q
                (j on partitions), E = exp(S^T), then O(c,i) accumulates
                with V^T slices stationary and E moving; the denominator
                row comes from an M=1 ones matmul.  The S/exp stage is
                emitted two key-tiles ahead of AV/den so the in-order PE
                queue never stalls on the ACT exp.  epilogue(ich, accs,
                rrep) gets natural-layout unnormalized O accumulators and
                the partition-replicated reciprocal denominator (SBUF)."""
                LOOKAHEAD = 2
                for ich in range(R // IC):
                    is_ = slice(ich * IC, (ich + 1) * IC)
                    accs = [
                        psO.tile([P, IC], f32, tag="o", name="acc")
                        for _ in range(2)
                    ]
                    den = psA.tile([1, IC], f32, tag="a", name="den")
                    es = {}

                    def s_exp(t):
                        sps = psS.tile([P, IC], f32, tag="s", name="s_ps")
                        nc.tensor.matmul(
                            sps[:],
                            k_sb[:, t * P : (t + 1) * P],
                            q_sb[:, is_],
                            start=True, stop=True,
                        )
                        e_sb = epool.tile([P, IC], bf16, tag="e", name="e_sb")
                        nc.scalar.activation(e_sb[:], sps[:], AF.Exp)
                        es[t] = e_sb

                    for t in range(LOOKAHEAD):
                        s_exp(t)
                    for t in range(NT):
                        if t + LOOKAHEAD < NT:
                            s_exp(t + LOOKAHEAD)
                        e_sb = es.pop(t)
                        for ct in range(2):
                            nc.tensor.matmul(
                                accs[ct][:],
                                v_sb[:, t, ct * P : (ct + 1) * P],
                                e_sb[:],
                                start=(t == 0), stop=(t == NT - 1),
                            )
                        nc.tensor.matmul(
                            den[:], onesc_sb[:], e_sb[:],
                            start=(t == 0), stop=(t == NT - 1),
                        )
                    # reciprocal of the denominator row, replicated to all
                    # partitions via a K=1 ones matmul (f32r: single-pass)
                    rrow = rcpool.tile([1, IC], f32, tag="rc", name="rrow")
                    nc.vector.reciprocal(rrow[:], den[:])
                    rrow_r = rcpool.tile([1, IC], f32r, tag="rcr", name="rrow_r")
                    nc.vector.tensor_copy(rrow_r[:], rrow[:])
                    rrep_ps = psA.tile([P, IC], f32, tag="a", name="rrep_ps")
                    nc.tensor.matmul(
                        rrep_ps[:], onesr_sb[:], rrow_r[:],
                        start=True, stop=True,
                    )
                    rrep = onpool.tile([P, IC], f32, tag="rr", name="rrep")
                    nc.scalar.copy(rrep[:], rrep_ps[:])
                    epilogue(ich, accs, rrep)

            # ================= Layer 1: self-attention =================
            q1_sb = big.tile([D, R], bf16, tag="q", name="q1_sb")
            k1_sb = big.tile([D, N], bf16, tag="k", name="k1_sb")
            v1_sb = big.tile([P, NT, C], bf16, tag="v", name="v1_sb")

            conv_qk(wqT1_sb, 6, lambda k, js: xc_sb[:, k, js], R, q1_sb)
            conv_qk(wkT1_sb, 7, lambda k, js: xf_sb[:, k, js], N, k1_sb)
            for t in range(NT):
                conv_vT(wvT1_sb, lambda k, ts_: xf_sb[:, k, ts_], v1_sb, t)

            def epilogue1(ich, accs, rrep):
                io = slice(ich * IC, (ich + 1) * IC)
                for ct in range(2):
                    # x' = sa_gamma * (O/den) + sa_gamma*bv + x
                    nc.vector.tensor_mul(
                        xp_sb[:, ct, io], accs[ct][:], rrep[:]
                    )
                    nc.vector.tensor_scalar(
                        xp_sb[:, ct, io], xp_sb[:, ct, io],
                        consts_sb[:, 0:1], consts_sb[:, 2 + ct : 3 + ct],
                        op0=OP.mult, op1=OP.add,
                    )
                    nc.vector.tensor_add(
                        xp_sb[:, ct, io],
                        xp_sb[:, ct, io],
                        xc_sb[:, ct, io].bitcast(f32),
                    )

            attention(q1_sb, k1_sb, v1_sb, epilogue1)

            # ====== AllGather x' within each batch group (2 phases) ======
            # Phase h gathers x' columns [h*512, (h+1)*512) of every rank;
            # phase 0 overlaps the second layer-1 attention i-chunk.
            xpf_sb = big.tile([P, 2, N], f32, tag="xbig", name="xpf_sb")
            for h in range(2):
                hs = slice(h * IC, (h + 1) * IC)
                ag_in = dram.tile(
                    [C, IC], f32, tag=f"ag_in{h}", name=f"ag_in{h}"
                )
                ag_out = dram.tile(
                    [RSH, C, IC], f32, tag=f"ag_out{h}", name=f"ag_out{h}"
                )
                for ct in range(2):
                    nc.sync.dma_start(
                        out=ag_in[ct * P : (ct + 1) * P, :],
                        in_=xp_sb[:, ct, hs],
                    )
                nc.gpsimd.collective_compute(
                    "AllGather",
                    OP.bypass,
                    replica_groups=groups,
                    ins=[ag_in[:].opt()],
                    outs=[ag_out[:].opt()],
                )
                for ct in range(2):
                    for r in range(RSH):
                        nc.sync.dma_start(
                            out=xpf_sb[
                                :, ct, r * R + h * IC : r * R + (h + 1) * IC
                            ],
                            in_=ag_out[r, ct * P : (ct + 1) * P, :],
                        )

            # ============== Layer 2: masked cross-attention ==============
            # feature_f chunk + its per-channel stats (cols 0-3 of stats_sb)
            ffsq = misc.tile([P, R], f32, tag="ffsq", name="ffsq")
            for ct in range(2):
                nc.vector.tensor_mul(
                    ff_sb[:, ct, :], maskc_sb[:], xp_sb[:, ct, :]
                )
                nc.vector.tensor_reduce(
                    stats_sb[:, ct : ct + 1], ff_sb[:, ct, :].bitcast(f32),
                    axis=AX.X, op=OP.add,
                )
                nc.vector.tensor_mul(
                    ffsq[:],
                    ff_sb[:, ct, :].bitcast(f32),
                    ff_sb[:, ct, :].bitcast(f32),
                )
                nc.vector.tensor_reduce(
                    stats_sb[:, 2 + ct : 3 + ct], ffsq[:],
                    axis=AX.X, op=OP.add,
                )

            q2_sb = big.tile([D, R], bf16, tag="q", name="q2_sb")
            conv_qk(wqT2_sb, 8, lambda k, js: ff_sb[:, k, js], R, q2_sb)

            # feature_b tiles on the fly -> K2 and V2^T convs.  Chunk order
            # interleaves gather phases: even chunks only need AG phase 0.
            k2_sb = big.tile([D, N], bf16, tag="k", name="k2_sb")
            v2_sb = big.tile([P, NT, C], bf16, tag="v", name="v2_sb")
            for jc in (0, 2, 4, 6, 1, 3, 5, 7):
                js = slice(jc * IC, (jc + 1) * IC)
                fb0 = fbpool.tile([P, IC], bf16, tag="fb", name="fb0")
                fb1 = fbpool.tile([P, IC], bf16, tag="fb", name="fb1")
                # fb = (1-mask)*x' = x' - mask*x'
                nc.vector.tensor_mul(
                    fb0[:], mask_sb[:, js], xpf_sb[:, 0, js]
                )
                nc.vector.tensor_sub(
                    fb0[:], xpf_sb[:, 0, js], fb0[:]
                )
                nc.vector.tensor_mul(
                    fb1[:], mask_sb[:, js], xpf_sb[:, 1, js]
                )
                nc.vector.tensor_sub(
                    fb1[:], xpf_sb[:, 1, js], fb1[:]
                )
                ps = psA.tile([D, IC], f32, tag="a", name="k2_ps")
                nc.tensor.matmul(
                    ps[:], wkT2_sb[:, 0, :], fb0[:], start=True, stop=False
                )
                nc.tensor.matmul(
                    ps[:], wkT2_sb[:, 1, :], fb1[:], start=False, stop=True
                )
                nc.vector.tensor_scalar_add(
                    k2_sb[:, js], ps[:], consts_sb[0:D, 9:10]
                )
                for tsub in range(IC // P):
                    t = jc * (IC // P) + tsub
                    ts_ = slice(tsub * P, (tsub + 1) * P)
                    psv = psA.tile([P, C], f32, tag="a", name="v2_ps")
                    nc.tensor.matmul(
                        psv[:], fb0[:, ts_], wvT2_sb[:, 0, :],
                        start=True, stop=False,
                    )
                    nc.tensor.matmul(
                        psv[:], fb1[:, ts_], wvT2_sb[:, 1, :],
                        start=False, stop=True,
                    )
                    nc.vector.tensor_copy(v2_sb[:, t, :], psv[:])

            def epilogue2(ich, accs, rrep):
                # normalized sw_bg chunk in natural layout; accumulate
                # per-channel sum/sumsq into stats_sb cols 4-7 via VE
                for ct in range(2):
                    onb = onpool.tile([P, IC], f32, tag="on", name="on2")
                    nc.vector.tensor_mul(onb[:], accs[ct][:], rrep[:])
                    s1 = rcpool.tile([P, 1], f32, tag="s1", name="s1")
                    nc.vector.tensor_reduce(
                        s1[:], onb[:], axis=AX.X, op=OP.add
                    )
                    sqb = sqpool.tile([P, IC], f32, tag="sq", name="sq2")
                    nc.vector.tensor_mul(sqb[:], onb[:], onb[:])
                    s2 = rcpool.tile([P, 1], f32, tag="s2", name="s2")
                    nc.vector.tensor_reduce(
                        s2[:], sqb[:], axis=AX.X, op=OP.add
                    )
                    if ich == 0:
                        nc.vector.tensor_copy(
                            stats_sb[:, 4 + ct : 5 + ct], s1[:]
                        )
                        nc.vector.tensor_copy(
                            stats_sb[:, 6 + ct : 7 + ct], s2[:]
                        )
                    else:
                        nc.vector.tensor_add(
                            stats_sb[:, 4 + ct : 5 + ct],
                            stats_sb[:, 4 + ct : 5 + ct], s1[:],
                        )
                        nc.vector.tensor_add(
                            stats_sb[:, 6 + ct : 7 + ct],
                            stats_sb[:, 6 + ct : 7 + ct], s2[:],
                        )

            attention(q2_sb, k2_sb, v2_sb, epilogue2)

            # ================== stats AllReduce + FMM ==================
            ar_in = dram.tile([P, 8], f32, tag="ar_in", name="ar_in")
            ar_out = dram.tile([P, 8], f32, tag="ar_out", name="ar_out")
            nc.sync.dma_start(out=ar_in[:], in_=stats_sb[:])
            nc.gpsimd.collective_compute(
                "AllReduce",
                OP.add,
                replica_groups=groups,
                ins=[ar_in[:].opt()],
                outs=[ar_out[:].opt()],
            )
            rst = misc.tile([P, 8], f32, tag="rst", name="rst")
            nc.sync.dma_start(out=rst[:], in_=ar_out[:])

            # var = (S2 - S1^2/N)/(N-1) + EPS (both channel-halves at once)
            varf = misc.tile([P, 2], f32, tag="varf", name="varf")
            varg = misc.tile([P, 2], f32, tag="varg", name="varg")
            ratio = misc.tile([P, 2], f32, tag="ratio", name="ratio")
            for var, s1s, s2s in ((varf, 0, 2), (varg, 4, 6)):
                nc.vector.tensor_mul(
                    var[:], rst[:, s1s : s1s + 2], rst[:, s1s : s1s + 2]
                )
                nc.vector.tensor_scalar(
                    var[:], var[:], -1.0 / N, None, op0=OP.mult
                )
                nc.vector.tensor_add(var[:], var[:], rst[:, s2s : s2s + 2])
                nc.vector.tensor_scalar(
                    var[:], var[:], 1.0 / (N - 1), EPS, op0=OP.mult, op1=OP.add
                )
            nc.vector.reciprocal(varf[:], varf[:])
            nc.vector.tensor_mul(varg[:], varg[:], varf[:])
            nc.scalar.activation(ratio[:], varg[:], AF.Sqrt)
            # fold in gamma
            nc.vector.tensor_scalar_mul(ratio[:], ratio[:], consts_sb[:, 1:2])

            # out = x' + (gamma * std_bg/std_f) * ff
            for ct in range(2):
                fin = finpool.tile([P, R], f32, tag="fin", name="fin")
                nc.vector.tensor_scalar_mul(
                    fin[:], ff_sb[:, ct, :].bitcast(f32),
                    ratio[:, ct : ct + 1],
                )
                nc.vector.tensor_add(fin[:], fin[:], xp_sb[:, ct, :])
                nc.sync.dma_start(
                    out=out_d[ct * P : (ct + 1) * P, :], in_=fin[:]
                )

    nc.compile()
    return nc


def _prep_inputs(x, mask, sa_wq, sa_bq, sa_wk, sa_bk, sa_wv, sa_bv, sa_gamma,
                 wq, bq, wk, bk, wv, bv, gamma):
    """Build the per-core input maps (host-side sharding + weight layout)."""
    x = np.ascontiguousarray(x, dtype=F32)
    mask = np.ascontiguousarray(mask, dtype=F32)

    wqT1 = np.ascontiguousarray(sa_wq.T, dtype=F32)
    wkT1 = np.ascontiguousarray(sa_wk.T, dtype=F32)
    wvT1 = np.ascontiguousarray(sa_wv.T, dtype=F32)
    wqT2 = np.ascontiguousarray(wq.T, dtype=F32)
    wkT2 = np.ascontiguousarray(wk.T, dtype=F32)
    wvT2 = np.ascontiguousarray(wv.T, dtype=F32)

    consts = np.zeros((P, 10), dtype=F32)
    consts[:, 0] = sa_gamma[0]
    consts[:, 1] = gamma[0]
    sgb = (sa_gamma[0] * sa_bv).astype(F32)
    consts[:, 2] = sgb[0:P]
    consts[:, 3] = sgb[P:C]
    consts[0:D, 6] = sa_bq
    consts[0:D, 7] = sa_bk
    consts[0:D, 8] = bq
    consts[0:D, 9] = bk

    in_maps = []
    for g in range(NCORES):
        b, r = g // RSH, g % RSH
        xb = np.ascontiguousarray(x[b].reshape(C, N))
        mb = np.ascontiguousarray(mask[b].reshape(1, N))
        in_maps.append({
            "xf": xb,
            "xc": np.ascontiguousarray(xb[:, r * R : (r + 1) * R]),
            "mrow": mb,
            "mcrow": np.ascontiguousarray(mb[:, r * R : (r + 1) * R]),
            "wqT1": wqT1, "wkT1": wkT1, "wvT1": wvT1,
            "wqT2": wqT2, "wkT2": wkT2, "wvT2": wvT2,
            "consts": consts,
        })
    return in_maps


def kernel(**inputs):
    from concourse import bass_utils

    if "nc" not in _CACHE:
        _CACHE["nc"] = _build_bass()
    nc = _CACHE["nc"]

    in_maps = _prep_inputs(**inputs)
    res = bass_utils.run_bass_kernel_spmd(
        nc, in_maps, core_ids=list(range(NCORES))
    )
    _CACHE["last_results"] = res

    out = np.empty((B, C, N), dtype=F32)
    for g in range(NCORES):
        b, r = g // RSH, g % RSH
        out[b, :, r * R : (r + 1) * R] = res.results[g]["outc"]
    return out.reshape(B, C, HH, WW)


# revision 15
# speedup vs baseline: 1.2413x; 1.0381x over previous
"""Trainium2 Bass/Tile kernel for nn_FB_FMM (sparse_attention).

Computation (per batch element b, with N = H*W = 4096 tokens, C=256, D=32):
  1. Self-attention:  sa_out = attn(conv(x,sa_wq), conv(x,sa_wk), conv(x,sa_wv))
     x' = sa_gamma * sa_out + x
  2. Masked cross-attention (FB_FMM):
     ff = mask * x'; fb = (1-mask) * x'
     sw_bg = attn(conv(ff,wq), conv(fb,wk), conv(fb,wv))
     out = x' + gamma * ff * (std(sw_bg)/std(ff))    [per-channel std, ddof=1]

Sharding: 8 cores = 2 batch groups x 4-way query-row sharding (1024 rows each).
Each core computes its row-chunk of both attention layers; K/V sides are
computed redundantly per core (cheap: D=32 / one C x C conv). Cross-core
communication inside the kernel:
  - AllGather of x' chunks within each 4-core batch group (layer-2 K/V need
    the full x'), split into two 512-row phases so the first overlaps the
    second half of the layer-1 attention loop.
  - AllReduce of per-channel [sum, sumsq] stats for the FMM std ratio.

Layouts: feature maps are channel-major (C on partitions). Scores are computed
transposed (S^T: keys j on partitions, queries i free; logits are small so exp
needs no max-subtraction pass). The AV matmul keeps V^T slices stationary
(weight reuse) with E^T moving, producing O in natural (c x i) layout; the
softmax denominator comes from one extra M=1 ones-matmul per tile, and the
reciprocal row is broadcast across partitions with a K=1 ones matmul. All
heavy matmuls run in float32r (1 cycle/row vs 4 for fp32; ~1e-4 relative
rounding). V-conv biases are folded out mathematically (sum_j A[i,j] = 1
makes the layer-1 V bias a constant shift folded into the residual; variance
is shift-invariant so the layer-2 V bias drops out of the FMM std).
"""

import numpy as np

P = 128
B, C, HH, WW = 2, 256, 64, 64
N = HH * WW            # 4096 tokens
D = 32                 # q/k channels
NCORES = 8
RSH = 4                # row shards per batch group
R = N // RSH           # 1024 query rows per core
NT = N // P            # 32 key tiles
IC = 512               # query i-chunk (one PSUM bank of fp32)
EPS = 1e-5
F32 = np.float32

_CACHE = {}


def _build_bass():
    """Build the Bass/Tile program (single SPMD NEFF for all 8 cores)."""
    import concourse.bass as bass
    from concourse import bacc, mybir, tile

    f32 = mybir.dt.float32
    f32r = mybir.dt.float32r
    bf16 = mybir.dt.bfloat16
    AX = mybir.AxisListType
    OP = mybir.AluOpType
    AF = mybir.ActivationFunctionType

    nc = bacc.Bacc(
        "TRN2", target_bir_lowering=False, debug=False, num_devices=NCORES
    )
    bf16d = mybir.dt.bfloat16

    # ---------------- I/O ----------------
    xf_d = nc.dram_tensor("xf", [C, N], bf16d, kind="ExternalInput")
    xc_d = nc.dram_tensor("xc", [C, R], f32r, kind="ExternalInput")
    mrow_d = nc.dram_tensor("mrow", [1, N], f32, kind="ExternalInput")
    mcrow_d = nc.dram_tensor("mcrow", [1, R], f32, kind="ExternalInput")
    wqT1_d = nc.dram_tensor("wqT1", [C, D], f32r, kind="ExternalInput")
    wkT1_d = nc.dram_tensor("wkT1", [C, D], bf16d, kind="ExternalInput")
    wvT1_d = nc.dram_tensor("wvT1", [C, C], bf16d, kind="ExternalInput")
    wqT2_d = nc.dram_tensor("wqT2", [C, D], f32r, kind="ExternalInput")
    wkT2_d = nc.dram_tensor("wkT2", [C, D], bf16d, kind="ExternalInput")
    wvT2_d = nc.dram_tensor("wvT2", [C, C], bf16d, kind="ExternalInput")
    # consts columns: 0 sa_gamma, 1 gamma, 2/3 sa_gamma*sa_bv halves,
    # 6 sa_bq, 7 sa_bk, 8 bq, 9 bk (cols 6-9 live on partitions 0..31)
    consts_d = nc.dram_tensor("consts", [P, 10], f32, kind="ExternalInput")
    out_d = nc.dram_tensor("outc", [C, R], f32, kind="ExternalOutput")

    groups = [[0, 1, 2, 3], [4, 5, 6, 7]]

    with tile.TileContext(nc) as tc:
        from contextlib import ExitStack

        ctx = ExitStack()
        with ctx:
            big = ctx.enter_context(tc.tile_pool(name="big", bufs=1))
            epool = ctx.enter_context(tc.tile_pool(name="epool", bufs=4))
            onpool = ctx.enter_context(tc.tile_pool(name="onpool", bufs=3))
            sqpool = ctx.enter_context(tc.tile_pool(name="sqpool", bufs=2))
            fbpool = ctx.enter_context(tc.tile_pool(name="fbpool", bufs=4))
            rcpool = ctx.enter_context(tc.tile_pool(name="rcpool", bufs=4))
            finpool = ctx.enter_context(tc.tile_pool(name="finpool", bufs=2))
            misc = ctx.enter_context(tc.tile_pool(name="misc", bufs=1))
            psA = ctx.enter_context(
                tc.tile_pool(name="psA", bufs=2, space="PSUM")
            )
            psS = ctx.enter_context(
                tc.tile_pool(name="psS", bufs=3, space="PSUM")
            )
            psO = ctx.enter_context(
                tc.tile_pool(name="psO", bufs=3, space="PSUM")
            )
            dram = ctx.enter_context(
                tc.tile_pool(name="dram", bufs=1, space="DRAM")
            )

            # ------------- persistent SBUF tiles -------------
            xc_sb = big.tile([P, 2, R], f32r, tag="xc", name="xc_sb")
            mask_sb = big.tile([P, N], f32, tag="mask", name="mask_sb")
            maskc_sb = big.tile([P, R], f32, tag="maskc", name="maskc_sb")
            xp_sb = big.tile([P, 2, R], f32, tag="xp", name="xp_sb")
            xp16_sb = big.tile([P, 2, R], bf16, tag="xp16", name="xp16_sb")
            ff_sb = big.tile([P, 2, R], f32r, tag="ff", name="ff_sb")
            wqT1_sb = big.tile([P, 2, D], f32r, tag="wqT1", name="wqT1_sb")
            wkT1_sb = big.tile([P, 2, D], bf16, tag="wkT1", name="wkT1_sb")
            wvT1_sb = big.tile([P, 2, C], bf16, tag="wvT1", name="wvT1_sb")
            wqT2_sb = big.tile([P, 2, D], f32r, tag="wqT2", name="wqT2_sb")
            wkT2_sb = big.tile([P, 2, D], bf16, tag="wkT2", name="wkT2_sb")
            wvT2_sb = big.tile([P, 2, C], bf16, tag="wvT2", name="wvT2_sb")
            consts_sb = big.tile([P, 10], f32, tag="consts", name="consts_sb")
            # ones column (f32r) for the denominator matmul; ones row (f32)
            # for the K=1 reciprocal-replication matmul
            onesc_sb = big.tile([P, 1], bf16, tag="onesc", name="onesc_sb")
            onesr_sb = big.tile([1, P], f32r, tag="onesr", name="onesr_sb")
            stats_sb = misc.tile([P, 8], f32, tag="stats", name="stats_sb")

            # input DMAs: small tensors first on the HWDGE queue; x-full
            # and the bf16 weight casts go through gpsimd (casting DMAs run
            # on a separate queue and overlap)
            xf_sb = big.tile([P, 2, N], bf16, tag="xbig", name="xf_sb")
            nc.sync.dma_start(out=consts_sb[:], in_=consts_d[:])
            for k in range(2):
                cs = slice(k * P, (k + 1) * P)
                nc.sync.dma_start(out=wqT1_sb[:, k, :], in_=wqT1_d[cs, :])
                nc.sync.dma_start(out=wkT1_sb[:, k, :], in_=wkT1_d[cs, :])
                nc.sync.dma_start(out=wvT1_sb[:, k, :], in_=wvT1_d[cs, :])
                nc.sync.dma_start(out=xc_sb[:, k, :], in_=xc_d[cs, :])
                nc.sync.dma_start(out=wqT2_sb[:, k, :], in_=wqT2_d[cs, :])
                nc.sync.dma_start(out=wkT2_sb[:, k, :], in_=wkT2_d[cs, :])
                nc.sync.dma_start(out=wvT2_sb[:, k, :], in_=wvT2_d[cs, :])
            nc.sync.dma_start(
                out=mask_sb[:], in_=mrow_d[0, :].partition_broadcast(P)
            )
            nc.sync.dma_start(
                out=maskc_sb[:], in_=mcrow_d[0, :].partition_broadcast(P)
            )
            for k in range(2):
                for jc in range(4):
                    js = slice(jc * (N // 4), (jc + 1) * (N // 4))
                    nc.sync.dma_start(
                        out=xf_sb[:, k, js],
                        in_=xf_d[k * P : (k + 1) * P, js],
                    )
            nc.vector.memset(onesc_sb[:], 1.0)
            nc.vector.memset(onesr_sb[:].bitcast(f32), 1.0)

            def conv_qk(wT_sb, bias_col, src_of, width, out_sb):
                """out (D x width) = wT.T @ src + bias.  src_of(k, js) gives
                the (128 x 512) input-channel tile."""
                for jc in range(width // IC):
                    js = slice(jc * IC, (jc + 1) * IC)
                    ps = psA.tile([D, IC], f32, tag="a", name="qk_ps")
                    nc.tensor.matmul(
                        ps[:], wT_sb[:, 0, :], src_of(0, js),
                        start=True, stop=False,
                    )
                    nc.tensor.matmul(
                        ps[:], wT_sb[:, 1, :], src_of(1, js),
                        start=False, stop=True,
                    )
                    nc.vector.tensor_scalar_add(
                        out_sb[:, js], ps[:],
                        consts_sb[0:D, bias_col : bias_col + 1],
                    )

            def conv_vT(wvT_sb, src_of, v_sb, t):
                """v_sb[:, t, :] = (src^T @ wvT) for key tile t (j on
                partitions, channels free)."""
                ts_ = slice(t * P, (t + 1) * P)
                ps = psA.tile([P, C], f32, tag="a", name="v_ps")
                nc.tensor.matmul(
                    ps[:], src_of(0, ts_), wvT_sb[:, 0, :],
                    start=True, stop=False,
                )
                nc.tensor.matmul(
                    ps[:], src_of(1, ts_), wvT_sb[:, 1, :],
                    start=False, stop=True,
                )
                nc.vector.tensor_copy(v_sb[:, t, :], ps[:])

            def attention(q_sb, k_sb, v_sb, epilogue):
                """Row-chunk attention.  Per i-chunk: S^T = K-tile^T Q
                (j on partitions), E = exp(S^T), then O(c,i) accumulates
                with V^T slices stationary and E moving; the denominator
                row comes from an M=1 ones matmul.  The S/exp stage is
                emitted two key-tiles ahead of AV/den so the in-order PE
                queue never stalls on the ACT exp.  epilogue(ich, accs,
                rrep) gets natural-layout unnormalized O accumulators and
                the partition-replicated reciprocal denominator (SBUF)."""
                LOOKAHEAD = 2
                for ich in range(R // IC):
                    is_ = slice(ich * IC, (ich + 1) * IC)
                    accs = [
                        psO.tile([P, IC], f32, tag="o", name="acc")
                        for _ in range(2)
                    ]
                    den = psA.tile([1, IC], f32, tag="a", name="den")
                    es = {}

                    def s_exp(t):
                        sps = psS.tile([P, IC], f32, tag="s", name="s_ps")
                        nc.tensor.matmul(
                            sps[:],
                            k_sb[:, t * P : (t + 1) * P],
                            q_sb[:, is_],
                            start=True, stop=True,
                        )
                        e_sb = epool.tile([P, IC], bf16, tag="e", name="e_sb")
                        nc.scalar.activation(e_sb[:], sps[:], AF.Exp)
                        es[t] = e_sb

                    for t in range(LOOKAHEAD):
                        s_exp(t)
                    for t in range(NT):
                        if t + LOOKAHEAD < NT:
                            s_exp(t + LOOKAHEAD)
                        e_sb = es.pop(t)
                        for ct in range(2):
                            nc.tensor.matmul(
                                accs[ct][:],
                                v_sb[:, t, ct * P : (ct + 1) * P],
                                e_sb[:],
                                start=(t == 0), stop=(t == NT - 1),
                            )
                        nc.tensor.matmul(
                            den[:], onesc_sb[:], e_sb[:],
                            start=(t == 0), stop=(t == NT - 1),
                        )
                    # reciprocal of the denominator row, replicated to all
                    # partitions via a K=1 ones matmul (f32r: single-pass)
                    rrow = rcpool.tile([1, IC], f32, tag="rc", name="rrow")
                    nc.vector.reciprocal(rrow[:], den[:])
                    rrow_r = rcpool.tile([1, IC], f32r, tag="rcr", name="rrow_r")
                    nc.vector.tensor_copy(rrow_r[:], rrow[:])
                    rrep_ps = psA.tile([P, IC], f32, tag="a", name="rrep_ps")
                    nc.tensor.matmul(
                        rrep_ps[:], onesr_sb[:], rrow_r[:],
                        start=True, stop=True,
                    )
                    rrep = onpool.tile([P, IC], f32, tag="rr", name="rrep")
                    nc.scalar.copy(rrep[:], rrep_ps[:])
                    epilogue(ich, accs, rrep)

            # ================= Layer 1: self-attention =================
            q1_sb = big.tile([D, R], bf16, tag="q", name="q1_sb")
            k1_sb = big.tile([D, N], bf16, tag="k", name="k1_sb")
            v1_sb = big.tile([P, NT, C], bf16, tag="v", name="v1_sb")

            conv_qk(wqT1_sb, 6, lambda k, js: xc_sb[:, k, js], R, q1_sb)
            conv_qk(wkT1_sb, 7, lambda k, js: xf_sb[:, k, js], N, k1_sb)
            for t in range(NT):
                conv_vT(wvT1_sb, lambda k, ts_: xf_sb[:, k, ts_], v1_sb, t)

            def epilogue1(ich, accs, rrep):
                io = slice(ich * IC, (ich + 1) * IC)
                for ct in range(2):
                    # x' = sa_gamma * (O/den) + sa_gamma*bv + x
                    nc.vector.tensor_mul(
                        xp_sb[:, ct, io], accs[ct][:], rrep[:]
                    )
                    nc.vector.tensor_scalar(
                        xp_sb[:, ct, io], xp_sb[:, ct, io],
                        consts_sb[:, 0:1], consts_sb[:, 2 + ct : 3 + ct],
                        op0=OP.mult, op1=OP.add,
                    )
                    nc.vector.tensor_add(
                        xp_sb[:, ct, io],
                        xp_sb[:, ct, io],
                        xc_sb[:, ct, io].bitcast(f32),
                    )
                    nc.vector.tensor_copy(
                        xp16_sb[:, ct, io], xp_sb[:, ct, io]
                    )

            attention(q1_sb, k1_sb, v1_sb, epilogue1)

            # ====== AllGather x' within each batch group (2 phases) ======
            # Phase h gathers x' columns [h*512, (h+1)*512) of every rank;
            # phase 0 overlaps the second layer-1 attention i-chunk.
            xpf_sb = big.tile([P, 2, N], bf16, tag="xbig", name="xpf_sb")
            for h in range(2):
                hs = slice(h * IC, (h + 1) * IC)
                ag_in = dram.tile(
                    [C, IC], bf16, tag=f"ag_in{h}", name=f"ag_in{h}"
                )
                ag_out = dram.tile(
                    [RSH, C, IC], bf16, tag=f"ag_out{h}", name=f"ag_out{h}"
                )
                for ct in range(2):
                    nc.sync.dma_start(
                        out=ag_in[ct * P : (ct + 1) * P, :],
                        in_=xp16_sb[:, ct, hs],
                    )
                nc.gpsimd.collective_compute(
                    "AllGather",
                    OP.bypass,
                    replica_groups=groups,
                    ins=[ag_in[:].opt()],
                    outs=[ag_out[:].opt()],
                )
                for ct in range(2):
                    for r in range(RSH):
                        nc.sync.dma_start(
                            out=xpf_sb[
                                :, ct, r * R + h * IC : r * R + (h + 1) * IC
                            ],
                            in_=ag_out[r, ct * P : (ct + 1) * P, :],
                        )

            # ============== Layer 2: masked cross-attention ==============
            # feature_f chunk + its per-channel stats (cols 0-3 of stats_sb)
            ffsq = misc.tile([P, R], f32, tag="ffsq", name="ffsq")
            for ct in range(2):
                nc.vector.tensor_mul(
                    ff_sb[:, ct, :], maskc_sb[:], xp_sb[:, ct, :]
                )
                nc.vector.tensor_reduce(
                    stats_sb[:, ct : ct + 1], ff_sb[:, ct, :].bitcast(f32),
                    axis=AX.X, op=OP.add,
                )
                nc.vector.tensor_mul(
                    ffsq[:],
                    ff_sb[:, ct, :].bitcast(f32),
                    ff_sb[:, ct, :].bitcast(f32),
                )
                nc.vector.tensor_reduce(
                    stats_sb[:, 2 + ct : 3 + ct], ffsq[:],
                    axis=AX.X, op=OP.add,
                )

            q2_sb = big.tile([D, R], bf16, tag="q", name="q2_sb")
            conv_qk(wqT2_sb, 8, lambda k, js: ff_sb[:, k, js], R, q2_sb)

            # feature_b tiles on the fly -> K2 and V2^T convs.  Chunk order
            # interleaves gather phases: even chunks only need AG phase 0.
            k2_sb = big.tile([D, N], bf16, tag="k", name="k2_sb")
            v2_sb = big.tile([P, NT, C], bf16, tag="v", name="v2_sb")
            for jc in (0, 2, 4, 6, 1, 3, 5, 7):
                js = slice(jc * IC, (jc + 1) * IC)
                fb0 = fbpool.tile([P, IC], bf16, tag="fb", name="fb0")
                fb1 = fbpool.tile([P, IC], bf16, tag="fb", name="fb1")
                # fb = (1-mask)*x' = x' - mask*x'
                nc.vector.tensor_mul(
                    fb0[:], mask_sb[:, js], xpf_sb[:, 0, js]
                )
                nc.vector.tensor_sub(
                    fb0[:], xpf_sb[:, 0, js], fb0[:]
                )
                nc.vector.tensor_mul(
                    fb1[:], mask_sb[:, js], xpf_sb[:, 1, js]
                )
                nc.vector.tensor_sub(
                    fb1[:], xpf_sb[:, 1, js], fb1[:]
                )
                ps = psA.tile([D, IC], f32, tag="a", name="k2_ps")
                nc.tensor.matmul(
                    ps[:], wkT2_sb[:, 0, :], fb0[:], start=True, stop=False
                )
                nc.tensor.matmul(
                    ps[:], wkT2_sb[:, 1, :], fb1[:], start=False, stop=True
                )
                nc.vector.tensor_scalar_add(
                    k2_sb[:, js], ps[:], consts_sb[0:D, 9:10]
                )
                for tsub in range(IC // P):
                    t = jc * (IC // P) + tsub
                    ts_ = slice(tsub * P, (tsub + 1) * P)
                    psv = psA.tile([P, C], f32, tag="a", name="v2_ps")
                    nc.tensor.matmul(
                        psv[:], fb0[:, ts_], wvT2_sb[:, 0, :],
                        start=True, stop=False,
                    )
                    nc.tensor.matmul(
                        psv[:], fb1[:, ts_], wvT2_sb[:, 1, :],
                        start=False, stop=True,
                    )
                    nc.vector.tensor_copy(v2_sb[:, t, :], psv[:])

            def epilogue2(ich, accs, rrep):
                # normalized sw_bg chunk in natural layout; accumulate
                # per-channel sum/sumsq into stats_sb cols 4-7 via VE
                for ct in range(2):
                    onb = onpool.tile([P, IC], f32, tag="on", name="on2")
                    nc.vector.tensor_mul(onb[:], accs[ct][:], rrep[:])
                    s1 = rcpool.tile([P, 1], f32, tag="s1", name="s1")
                    nc.vector.tensor_reduce(
                        s1[:], onb[:], axis=AX.X, op=OP.add
                    )
                    sqb = sqpool.tile([P, IC], f32, tag="sq", name="sq2")
                    nc.vector.tensor_mul(sqb[:], onb[:], onb[:])
                    s2 = rcpool.tile([P, 1], f32, tag="s2", name="s2")
                    nc.vector.tensor_reduce(
                        s2[:], sqb[:], axis=AX.X, op=OP.add
                    )
                    if ich == 0:
                        nc.vector.tensor_copy(
                            stats_sb[:, 4 + ct : 5 + ct], s1[:]
                        )
                        nc.vector.tensor_copy(
                            stats_sb[:, 6 + ct : 7 + ct], s2[:]
                        )
                    else:
                        nc.vector.tensor_add(
                            stats_sb[:, 4 + ct : 5 + ct],
                            stats_sb[:, 4 + ct : 5 + ct], s1[:],
                        )
                        nc.vector.tensor_add(
                            stats_sb[:, 6 + ct : 7 + ct],
                            stats_sb[:, 6 + ct : 7 + ct], s2[:],
                        )

            attention(q2_sb, k2_sb, v2_sb, epilogue2)

            # ================== stats AllReduce + FMM ==================
            ar_in = dram.tile([P, 8], f32, tag="ar_in", name="ar_in")
            ar_out = dram.tile([P, 8], f32, tag="ar_out", name="ar_out")
            nc.sync.dma_start(out=ar_in[:], in_=stats_sb[:])
            nc.gpsimd.collective_compute(
                "AllReduce",
                OP.add,
                replica_groups=groups,
                ins=[ar_in[:].opt()],
                outs=[ar_out[:].opt()],
            )
            rst = misc.tile([P, 8], f32, tag="rst", name="rst")
            nc.sync.dma_start(out=rst[:], in_=ar_out[:])

            # var = (S2 - S1^2/N)/(N-1) + EPS (both channel-halves at once)
            varf = misc.tile([P, 2], f32, tag="varf", name="varf")
            varg = misc.tile([P, 2], f32, tag="varg", name="varg")
            ratio = misc.tile([P, 2], f32, tag="ratio", name="ratio")
            for var, s1s, s2s in ((varf, 0, 2), (varg, 4, 6)):
                nc.vector.tensor_mul(
                    var[:], rst[:, s1s : s1s + 2], rst[:, s1s : s1s + 2]
                )
                nc.vector.tensor_scalar(
                    var[:], var[:], -1.0 / N, None, op0=OP.mult
                )
                nc.vector.tensor_add(var[:], var[:], rst[:, s2s : s2s + 2])
                nc.vector.tensor_scalar(
                    var[:], var[:], 1.0 / (N - 1), EPS, op0=OP.mult, op1=OP.add
                )
            nc.vector.reciprocal(varf[:], varf[:])
            nc.vector.tensor_mul(varg[:], varg[:], varf[:])
            nc.scalar.activation(ratio[:], varg[:], AF.Sqrt)
            # fold in gamma
            nc.vector.tensor_scalar_mul(ratio[:], ratio[:], consts_sb[:, 1:2])

            # out = x' + (gamma * std_bg/std_f) * ff
            for ct in range(2):
                fin = finpool.tile([P, R], f32, tag="fin", name="fin")
                nc.vector.tensor_scalar_mul(
                    fin[:], ff_sb[:, ct, :].bitcast(f32),
                    ratio[:, ct : ct + 1],
                )
                nc.vector.tensor_add(fin[:], fin[:], xp_sb[:, ct, :])
                nc.sync.dma_start(
                    out=out_d[ct * P : (ct + 1) * P, :], in_=fin[:]
                )

    nc.compile()
    return nc


def _prep_inputs(x, mask, sa_wq, sa_bq, sa_wk, sa_bk, sa_wv, sa_bv, sa_gamma,
                 wq, bq, wk, bk, wv, bv, gamma):
    """Build the per-core input maps (host-side sharding + weight layout)."""
    x = np.ascontiguousarray(x, dtype=F32)
    mask = np.ascontiguousarray(mask, dtype=F32)

    import ml_dtypes

    BF16 = ml_dtypes.bfloat16
    wqT1 = np.ascontiguousarray(sa_wq.T, dtype=F32)
    wkT1 = np.ascontiguousarray(sa_wk.T.astype(BF16))
    wvT1 = np.ascontiguousarray(sa_wv.T.astype(BF16))
    wqT2 = np.ascontiguousarray(wq.T, dtype=F32)
    wkT2 = np.ascontiguousarray(wk.T.astype(BF16))
    wvT2 = np.ascontiguousarray(wv.T.astype(BF16))

    consts = np.zeros((P, 10), dtype=F32)
    consts[:, 0] = sa_gamma[0]
    consts[:, 1] = gamma[0]
    sgb = (sa_gamma[0] * sa_bv).astype(F32)
    consts[:, 2] = sgb[0:P]
    consts[:, 3] = sgb[P:C]
    consts[0:D, 6] = sa_bq
    consts[0:D, 7] = sa_bk
    consts[0:D, 8] = bq
    consts[0:D, 9] = bk

    in_maps = []
    for g in range(NCORES):
        b, r = g // RSH, g % RSH
        xb = np.ascontiguousarray(x[b].reshape(C, N))
        mb = np.ascontiguousarray(mask[b].reshape(1, N))
        in_maps.append({
            "xf": np.ascontiguousarray(xb.astype(BF16)),
            "xc": np.ascontiguousarray(xb[:, r * R : (r + 1) * R]),
            "mrow": mb,
            "mcrow": np.ascontiguousarray(mb[:, r * R : (r + 1) * R]),
            "wqT1": wqT1, "wkT1": wkT1, "wvT1": wvT1,
            "wqT2": wqT2, "wkT2": wkT2, "wvT2": wvT2,
            "consts": consts,
        })
    return in_maps


def kernel(**inputs):
    from concourse import bass_utils

    if "nc" not in _CACHE:
        _CACHE["nc"] = _build_bass()
    nc = _CACHE["nc"]

    in_maps = _prep_inputs(**inputs)
    res = bass_utils.run_bass_kernel_spmd(
        nc, in_maps, core_ids=list(range(NCORES))
    )
    _CACHE["last_results"] = res

    out = np.empty((B, C, N), dtype=F32)
    for g in range(NCORES):
        b, r = g // RSH, g % RSH
        out[b, :, r * R : (r + 1) * R] = res.results[g]["outc"]
    return out.reshape(B, C, HH, WW)


# revision 16
# speedup vs baseline: 1.2646x; 1.0188x over previous
"""Trainium2 Bass/Tile kernel for nn_FB_FMM (sparse_attention).

Computation (per batch element b, with N = H*W = 4096 tokens, C=256, D=32):
  1. Self-attention:  sa_out = attn(conv(x,sa_wq), conv(x,sa_wk), conv(x,sa_wv))
     x' = sa_gamma * sa_out + x
  2. Masked cross-attention (FB_FMM):
     ff = mask * x'; fb = (1-mask) * x'
     sw_bg = attn(conv(ff,wq), conv(fb,wk), conv(fb,wv))
     out = x' + gamma * ff * (std(sw_bg)/std(ff))    [per-channel std, ddof=1]

Sharding: 8 cores = 2 batch groups x 4-way query-row sharding (1024 rows each).
Each core computes its row-chunk of both attention layers; K/V sides are
computed redundantly per core (cheap: D=32 / one C x C conv). Cross-core
communication inside the kernel:
  - AllGather of x' chunks within each 4-core batch group (layer-2 K/V need
    the full x'), split into two 512-row phases so the first overlaps the
    second half of the layer-1 attention loop.
  - AllReduce of per-channel [sum, sumsq] stats for the FMM std ratio.

Layouts: feature maps are channel-major (C on partitions). Scores are computed
transposed (S^T: keys j on partitions, queries i free; logits are small so exp
needs no max-subtraction pass). The AV matmul keeps V^T slices stationary
(weight reuse) with E^T moving, producing O in natural (c x i) layout; the
softmax denominator comes from one extra M=1 ones-matmul per tile, and the
reciprocal row is broadcast across partitions with a K=1 ones matmul. All
heavy matmuls run in float32r (1 cycle/row vs 4 for fp32; ~1e-4 relative
rounding). V-conv biases are folded out mathematically (sum_j A[i,j] = 1
makes the layer-1 V bias a constant shift folded into the residual; variance
is shift-invariant so the layer-2 V bias drops out of the FMM std).
"""

import numpy as np

P = 128
B, C, HH, WW = 2, 256, 64, 64
N = HH * WW            # 4096 tokens
D = 32                 # q/k channels
NCORES = 8
RSH = 4                # row shards per batch group
R = N // RSH           # 1024 query rows per core
NT = N // P            # 32 key tiles
IC = 512               # query i-chunk (one PSUM bank of fp32)
EPS = 1e-5
F32 = np.float32

_CACHE = {}


def _build_bass():
    """Build the Bass/Tile program (single SPMD NEFF for all 8 cores)."""
    import concourse.bass as bass
    from concourse import bacc, mybir, tile

    f32 = mybir.dt.float32
    f32r = mybir.dt.float32r
    bf16 = mybir.dt.bfloat16
    AX = mybir.AxisListType
    OP = mybir.AluOpType
    AF = mybir.ActivationFunctionType

    nc = bacc.Bacc(
        "TRN2", target_bir_lowering=False, debug=False, num_devices=NCORES
    )
    bf16d = mybir.dt.bfloat16

    # ---------------- I/O ----------------
    xf_d = nc.dram_tensor("xf", [C, N], bf16d, kind="ExternalInput")
    xc_d = nc.dram_tensor("xc", [C, R], f32r, kind="ExternalInput")
    mrow_d = nc.dram_tensor("mrow", [1, N], f32, kind="ExternalInput")
    mcrow_d = nc.dram_tensor("mcrow", [1, R], f32, kind="ExternalInput")
    wqT1_d = nc.dram_tensor("wqT1", [C, D], f32r, kind="ExternalInput")
    wkT1_d = nc.dram_tensor("wkT1", [C, D], bf16d, kind="ExternalInput")
    wvT1_d = nc.dram_tensor("wvT1", [C, C], bf16d, kind="ExternalInput")
    wqT2_d = nc.dram_tensor("wqT2", [C, D], f32r, kind="ExternalInput")
    wkT2_d = nc.dram_tensor("wkT2", [C, D], bf16d, kind="ExternalInput")
    wvT2_d = nc.dram_tensor("wvT2", [C, C], bf16d, kind="ExternalInput")
    # consts columns: 0 sa_gamma, 1 gamma, 2/3 sa_gamma*sa_bv halves,
    # 6 sa_bq, 7 sa_bk, 8 bq, 9 bk (cols 6-9 live on partitions 0..31)
    consts_d = nc.dram_tensor("consts", [P, 10], f32, kind="ExternalInput")
    out_d = nc.dram_tensor("outc", [C, R], f32, kind="ExternalOutput")

    groups = [[0, 1, 2, 3], [4, 5, 6, 7]]

    with tile.TileContext(nc) as tc:
        from contextlib import ExitStack

        ctx = ExitStack()
        with ctx:
            big = ctx.enter_context(tc.tile_pool(name="big", bufs=1))
            epool = ctx.enter_context(tc.tile_pool(name="epool", bufs=4))
            onpool = ctx.enter_context(tc.tile_pool(name="onpool", bufs=3))
            sqpool = ctx.enter_context(tc.tile_pool(name="sqpool", bufs=2))
            fbpool = ctx.enter_context(tc.tile_pool(name="fbpool", bufs=4))
            rcpool = ctx.enter_context(tc.tile_pool(name="rcpool", bufs=4))
            finpool = ctx.enter_context(tc.tile_pool(name="finpool", bufs=2))
            misc = ctx.enter_context(tc.tile_pool(name="misc", bufs=1))
            psA = ctx.enter_context(
                tc.tile_pool(name="psA", bufs=2, space="PSUM")
            )
            psS = ctx.enter_context(
                tc.tile_pool(name="psS", bufs=3, space="PSUM")
            )
            psO = ctx.enter_context(
                tc.tile_pool(name="psO", bufs=3, space="PSUM")
            )
            dram = ctx.enter_context(
                tc.tile_pool(name="dram", bufs=1, space="DRAM")
            )

            # ------------- persistent SBUF tiles -------------
            xc_sb = big.tile([P, 2, R], f32r, tag="xc", name="xc_sb")
            mask_sb = big.tile([P, N], f32, tag="mask", name="mask_sb")
            maskc_sb = big.tile([P, R], f32, tag="maskc", name="maskc_sb")
            xp_sb = big.tile([P, 2, R], f32, tag="xp", name="xp_sb")
            xp16_sb = big.tile([P, 2, R], bf16, tag="xp16", name="xp16_sb")
            ff_sb = big.tile([P, 2, R], f32r, tag="ff", name="ff_sb")
            wqT1_sb = big.tile([P, 2, D], f32r, tag="wqT1", name="wqT1_sb")
            wkT1_sb = big.tile([P, 2, D], bf16, tag="wkT1", name="wkT1_sb")
            wvT1_sb = big.tile([P, 2, C], bf16, tag="wvT1", name="wvT1_sb")
            wqT2_sb = big.tile([P, 2, D], f32r, tag="wqT2", name="wqT2_sb")
            wkT2_sb = big.tile([P, 2, D], bf16, tag="wkT2", name="wkT2_sb")
            wvT2_sb = big.tile([P, 2, C], bf16, tag="wvT2", name="wvT2_sb")
            consts_sb = big.tile([P, 10], f32, tag="consts", name="consts_sb")
            # ones column (f32r) for the denominator matmul; ones row (f32)
            # for the K=1 reciprocal-replication matmul
            onesc_sb = big.tile([P, 1], bf16, tag="onesc", name="onesc_sb")
            onesr_sb = big.tile([1, P], f32r, tag="onesr", name="onesr_sb")
            stats_sb = misc.tile([P, 8], f32, tag="stats", name="stats_sb")

            # input DMAs: small tensors first on the HWDGE queue; x-full
            # and the bf16 weight casts go through gpsimd (casting DMAs run
            # on a separate queue and overlap)
            xf_sb = big.tile([P, 2, N], bf16, tag="xbig", name="xf_sb")
            for k in range(2):
                for jc in range(4):
                    js = slice(jc * (N // 4), (jc + 1) * (N // 4))
                    nc.sync.dma_start(
                        out=xf_sb[:, k, js],
                        in_=xf_d[k * P : (k + 1) * P, js],
                    )
            nc.sync.dma_start(out=consts_sb[:], in_=consts_d[:])
            for k in range(2):
                cs = slice(k * P, (k + 1) * P)
                nc.sync.dma_start(out=wqT1_sb[:, k, :], in_=wqT1_d[cs, :])
                nc.sync.dma_start(out=wkT1_sb[:, k, :], in_=wkT1_d[cs, :])
                nc.sync.dma_start(out=wvT1_sb[:, k, :], in_=wvT1_d[cs, :])
                nc.sync.dma_start(out=xc_sb[:, k, :], in_=xc_d[cs, :])
                nc.sync.dma_start(out=wqT2_sb[:, k, :], in_=wqT2_d[cs, :])
                nc.sync.dma_start(out=wkT2_sb[:, k, :], in_=wkT2_d[cs, :])
                nc.sync.dma_start(out=wvT2_sb[:, k, :], in_=wvT2_d[cs, :])
            nc.sync.dma_start(
                out=mask_sb[:], in_=mrow_d[0, :].partition_broadcast(P)
            )
            nc.sync.dma_start(
                out=maskc_sb[:], in_=mcrow_d[0, :].partition_broadcast(P)
            )
            nc.vector.memset(onesc_sb[:], 1.0)
            nc.vector.memset(onesr_sb[:].bitcast(f32), 1.0)

            def conv_qk(wT_sb, bias_col, src_of, width, out_sb):
                """out (D x width) = wT.T @ src + bias.  src_of(k, js) gives
                the (128 x 512) input-channel tile."""
                for jc in range(width // IC):
                    js = slice(jc * IC, (jc + 1) * IC)
                    ps = psA.tile([D, IC], f32, tag="a", name="qk_ps")
                    nc.tensor.matmul(
                        ps[:], wT_sb[:, 0, :], src_of(0, js),
                        start=True, stop=False,
                    )
                    nc.tensor.matmul(
                        ps[:], wT_sb[:, 1, :], src_of(1, js),
                        start=False, stop=True,
                    )
                    nc.vector.tensor_scalar_add(
                        out_sb[:, js], ps[:],
                        consts_sb[0:D, bias_col : bias_col + 1],
                    )

            def conv_vT(wvT_sb, src_of, v_sb, t):
                """v_sb[:, t, :] = (src^T @ wvT) for key tile t (j on
                partitions, channels free)."""
                ts_ = slice(t * P, (t + 1) * P)
                ps = psA.tile([P, C], f32, tag="a", name="v_ps")
                nc.tensor.matmul(
                    ps[:], src_of(0, ts_), wvT_sb[:, 0, :],
                    start=True, stop=False,
                )
                nc.tensor.matmul(
                    ps[:], src_of(1, ts_), wvT_sb[:, 1, :],
                    start=False, stop=True,
                )
                nc.vector.tensor_copy(v_sb[:, t, :], ps[:])

            def attention(q_sb, k_sb, v_sb, epilogue):
                """Row-chunk attention.  Per i-chunk: S^T = K-tile^T Q
                (j on partitions), E = exp(S^T), then O(c,i) accumulates
                with V^T slices stationary and E moving; the denominator
                row comes from an M=1 ones matmul.  The S/exp stage is
                emitted two key-tiles ahead of AV/den so the in-order PE
                queue never stalls on the ACT exp.  epilogue(ich, accs,
                rrep) gets natural-layout unnormalized O accumulators and
                the partition-replicated reciprocal denominator (SBUF)."""
                LOOKAHEAD = 2
                for ich in range(R // IC):
                    is_ = slice(ich * IC, (ich + 1) * IC)
                    accs = [
                        psO.tile([P, IC], f32, tag="o", name="acc")
                        for _ in range(2)
                    ]
                    den = psA.tile([1, IC], f32, tag="a", name="den")
                    es = {}

                    def s_exp(t):
                        sps = psS.tile([P, IC], f32, tag="s", name="s_ps")
                        nc.tensor.matmul(
                            sps[:],
                            k_sb[:, t * P : (t + 1) * P],
                            q_sb[:, is_],
                            start=True, stop=True,
                        )
                        e_sb = epool.tile([P, IC], bf16, tag="e", name="e_sb")
                        nc.scalar.activation(e_sb[:], sps[:], AF.Exp)
                        es[t] = e_sb

                    for t in range(LOOKAHEAD):
                        s_exp(t)
                    for t in range(NT):
                        if t + LOOKAHEAD < NT:
                            s_exp(t + LOOKAHEAD)
                        e_sb = es.pop(t)
                        for ct in range(2):
                            nc.tensor.matmul(
                                accs[ct][:],
                                v_sb[:, t, ct * P : (ct + 1) * P],
                                e_sb[:],
                                start=(t == 0), stop=(t == NT - 1),
                            )
                        nc.tensor.matmul(
                            den[:], onesc_sb[:], e_sb[:],
                            start=(t == 0), stop=(t == NT - 1),
                        )
                    # reciprocal of the denominator row, replicated to all
                    # partitions via a K=1 ones matmul (f32r: single-pass)
                    rrow = rcpool.tile([1, IC], f32, tag="rc", name="rrow")
                    nc.vector.reciprocal(rrow[:], den[:])
                    rrow_r = rcpool.tile([1, IC], f32r, tag="rcr", name="rrow_r")
                    nc.vector.tensor_copy(rrow_r[:], rrow[:])
                    rrep_ps = psA.tile([P, IC], f32, tag="a", name="rrep_ps")
                    nc.tensor.matmul(
                        rrep_ps[:], onesr_sb[:], rrow_r[:],
                        start=True, stop=True,
                    )
                    rrep = onpool.tile([P, IC], f32, tag="rr", name="rrep")
                    nc.scalar.copy(rrep[:], rrep_ps[:])
                    epilogue(ich, accs, rrep)

            # ================= Layer 1: self-attention =================
            q1_sb = big.tile([D, R], bf16, tag="q", name="q1_sb")
            k1_sb = big.tile([D, N], bf16, tag="k", name="k1_sb")
            v1_sb = big.tile([P, NT, C], bf16, tag="v", name="v1_sb")

            conv_qk(wqT1_sb, 6, lambda k, js: xc_sb[:, k, js], R, q1_sb)
            conv_qk(wkT1_sb, 7, lambda k, js: xf_sb[:, k, js], N, k1_sb)
            for t in range(NT):
                conv_vT(wvT1_sb, lambda k, ts_: xf_sb[:, k, ts_], v1_sb, t)

            def epilogue1(ich, accs, rrep):
                io = slice(ich * IC, (ich + 1) * IC)
                for ct in range(2):
                    # x' = sa_gamma * (O/den) + sa_gamma*bv + x, fused as
                    # ((O * sa_gamma) * rrep), then ((t + sgb) + x)
                    nc.vector.scalar_tensor_tensor(
                        xp_sb[:, ct, io], accs[ct][:],
                        consts_sb[:, 0:1], rrep[:],
                        op0=OP.mult, op1=OP.mult,
                    )
                    nc.vector.scalar_tensor_tensor(
                        xp_sb[:, ct, io], xp_sb[:, ct, io],
                        consts_sb[:, 2 + ct : 3 + ct],
                        xc_sb[:, ct, io].bitcast(f32),
                        op0=OP.add, op1=OP.add,
                    )
                    nc.vector.tensor_copy(
                        xp16_sb[:, ct, io], xp_sb[:, ct, io]
                    )

            attention(q1_sb, k1_sb, v1_sb, epilogue1)

            # ====== AllGather x' within each batch group (2 phases) ======
            # Phase h gathers x' columns [h*512, (h+1)*512) of every rank;
            # phase 0 overlaps the second layer-1 attention i-chunk.
            xpf_sb = big.tile([P, 2, N], bf16, tag="xbig", name="xpf_sb")
            for h in range(2):
                hs = slice(h * IC, (h + 1) * IC)
                ag_in = dram.tile(
                    [C, IC], bf16, tag=f"ag_in{h}", name=f"ag_in{h}"
                )
                ag_out = dram.tile(
                    [RSH, C, IC], bf16, tag=f"ag_out{h}", name=f"ag_out{h}"
                )
                for ct in range(2):
                    nc.sync.dma_start(
                        out=ag_in[ct * P : (ct + 1) * P, :],
                        in_=xp16_sb[:, ct, hs],
                    )
                nc.gpsimd.collective_compute(
                    "AllGather",
                    OP.bypass,
                    replica_groups=groups,
                    ins=[ag_in[:].opt()],
                    outs=[ag_out[:].opt()],
                )
                for ct in range(2):
                    for r in range(RSH):
                        nc.sync.dma_start(
                            out=xpf_sb[
                                :, ct, r * R + h * IC : r * R + (h + 1) * IC
                            ],
                            in_=ag_out[r, ct * P : (ct + 1) * P, :],
                        )

            # ============== Layer 2: masked cross-attention ==============
            # feature_f chunk + its per-channel stats (cols 0-3 of stats_sb)
            ffsq = misc.tile([P, R], f32, tag="ffsq", name="ffsq")
            for ct in range(2):
                nc.vector.tensor_mul(
                    ff_sb[:, ct, :], maskc_sb[:], xp_sb[:, ct, :]
                )
                nc.vector.tensor_reduce(
                    stats_sb[:, ct : ct + 1], ff_sb[:, ct, :].bitcast(f32),
                    axis=AX.X, op=OP.add,
                )
                nc.vector.tensor_mul(
                    ffsq[:],
                    ff_sb[:, ct, :].bitcast(f32),
                    ff_sb[:, ct, :].bitcast(f32),
                )
                nc.vector.tensor_reduce(
                    stats_sb[:, 2 + ct : 3 + ct], ffsq[:],
                    axis=AX.X, op=OP.add,
                )

            q2_sb = big.tile([D, R], bf16, tag="q", name="q2_sb")
            conv_qk(wqT2_sb, 8, lambda k, js: ff_sb[:, k, js], R, q2_sb)

            # feature_b tiles on the fly -> K2 and V2^T convs.  Chunk order
            # interleaves gather phases: even chunks only need AG phase 0.
            k2_sb = big.tile([D, N], bf16, tag="k", name="k2_sb")
            v2_sb = big.tile([P, NT, C], bf16, tag="v", name="v2_sb")
            for jc in (0, 2, 4, 6, 1, 3, 5, 7):
                js = slice(jc * IC, (jc + 1) * IC)
                fb0 = fbpool.tile([P, IC], bf16, tag="fb", name="fb0")
                fb1 = fbpool.tile([P, IC], bf16, tag="fb", name="fb1")
                # fb = (1-mask)*x' = x' - mask*x'
                nc.vector.tensor_mul(
                    fb0[:], mask_sb[:, js], xpf_sb[:, 0, js]
                )
                nc.vector.tensor_sub(
                    fb0[:], xpf_sb[:, 0, js], fb0[:]
                )
                nc.vector.tensor_mul(
                    fb1[:], mask_sb[:, js], xpf_sb[:, 1, js]
                )
                nc.vector.tensor_sub(
                    fb1[:], xpf_sb[:, 1, js], fb1[:]
                )
                ps = psA.tile([D, IC], f32, tag="a", name="k2_ps")
                nc.tensor.matmul(
                    ps[:], wkT2_sb[:, 0, :], fb0[:], start=True, stop=False
                )
                nc.tensor.matmul(
                    ps[:], wkT2_sb[:, 1, :], fb1[:], start=False, stop=True
                )
                nc.vector.tensor_scalar_add(
                    k2_sb[:, js], ps[:], consts_sb[0:D, 9:10]
                )
                for tsub in range(IC // P):
                    t = jc * (IC // P) + tsub
                    ts_ = slice(tsub * P, (tsub + 1) * P)
                    psv = psA.tile([P, C], f32, tag="a", name="v2_ps")
                    nc.tensor.matmul(
                        psv[:], fb0[:, ts_], wvT2_sb[:, 0, :],
                        start=True, stop=False,
                    )
                    nc.tensor.matmul(
                        psv[:], fb1[:, ts_], wvT2_sb[:, 1, :],
                        start=False, stop=True,
                    )
                    nc.vector.tensor_copy(v2_sb[:, t, :], psv[:])

            def epilogue2(ich, accs, rrep):
                # normalized sw_bg chunk in natural layout; accumulate
                # per-channel sum/sumsq into stats_sb cols 4-7 via VE
                for ct in range(2):
                    onb = onpool.tile([P, IC], f32, tag="on", name="on2")
                    s1 = rcpool.tile([P, 1], f32, tag="s1", name="s1")
                    nc.vector.scalar_tensor_tensor(
                        onb[:], accs[ct][:], 1.0, rrep[:],
                        op0=OP.mult, op1=OP.mult, accum_out=s1[:],
                    )
                    sqb = sqpool.tile([P, IC], f32, tag="sq", name="sq2")
                    s2 = rcpool.tile([P, 1], f32, tag="s2", name="s2")
                    nc.vector.scalar_tensor_tensor(
                        sqb[:], onb[:], 1.0, onb[:],
                        op0=OP.mult, op1=OP.mult, accum_out=s2[:],
                    )
                    if ich == 0:
                        nc.vector.tensor_copy(
                            stats_sb[:, 4 + ct : 5 + ct], s1[:]
                        )
                        nc.vector.tensor_copy(
                            stats_sb[:, 6 + ct : 7 + ct], s2[:]
                        )
                    else:
                        nc.vector.tensor_add(
                            stats_sb[:, 4 + ct : 5 + ct],
                            stats_sb[:, 4 + ct : 5 + ct], s1[:],
                        )
                        nc.vector.tensor_add(
                            stats_sb[:, 6 + ct : 7 + ct],
                            stats_sb[:, 6 + ct : 7 + ct], s2[:],
                        )

            attention(q2_sb, k2_sb, v2_sb, epilogue2)

            # ================== stats AllReduce + FMM ==================
            ar_in = dram.tile([P, 8], f32, tag="ar_in", name="ar_in")
            ar_out = dram.tile([P, 8], f32, tag="ar_out", name="ar_out")
            nc.sync.dma_start(out=ar_in[:], in_=stats_sb[:])
            nc.gpsimd.collective_compute(
                "AllReduce",
                OP.add,
                replica_groups=groups,
                ins=[ar_in[:].opt()],
                outs=[ar_out[:].opt()],
            )
            rst = misc.tile([P, 8], f32, tag="rst", name="rst")
            nc.sync.dma_start(out=rst[:], in_=ar_out[:])

            # var = (S2 - S1^2/N)/(N-1) + EPS (both channel-halves at once)
            varf = misc.tile([P, 2], f32, tag="varf", name="varf")
            varg = misc.tile([P, 2], f32, tag="varg", name="varg")
            ratio = misc.tile([P, 2], f32, tag="ratio", name="ratio")
            for var, s1s, s2s in ((varf, 0, 2), (varg, 4, 6)):
                nc.vector.tensor_mul(
                    var[:], rst[:, s1s : s1s + 2], rst[:, s1s : s1s + 2]
                )
                nc.vector.tensor_scalar(
                    var[:], var[:], -1.0 / N, None, op0=OP.mult
                )
                nc.vector.tensor_add(var[:], var[:], rst[:, s2s : s2s + 2])
                nc.vector.tensor_scalar(
                    var[:], var[:], 1.0 / (N - 1), EPS, op0=OP.mult, op1=OP.add
                )
            nc.vector.reciprocal(varf[:], varf[:])
            nc.vector.tensor_mul(varg[:], varg[:], varf[:])
            nc.scalar.activation(ratio[:], varg[:], AF.Sqrt)
            # fold in gamma
            nc.vector.tensor_scalar_mul(ratio[:], ratio[:], consts_sb[:, 1:2])

            # out = x' + (gamma * std_bg/std_f) * ff
            for ct in range(2):
                fin = finpool.tile([P, R], f32, tag="fin", name="fin")
                nc.vector.scalar_tensor_tensor(
                    fin[:], ff_sb[:, ct, :].bitcast(f32),
                    ratio[:, ct : ct + 1], xp_sb[:, ct, :],
                    op0=OP.mult, op1=OP.add,
                )
                nc.sync.dma_start(
                    out=out_d[ct * P : (ct + 1) * P, :], in_=fin[:]
                )

    nc.compile()
    return nc


def _prep_inputs(x, mask, sa_wq, sa_bq, sa_wk, sa_bk, sa_wv, sa_bv, sa_gamma,
                 wq, bq, wk, bk, wv, bv, gamma):
    """Build the per-core input maps (host-side sharding + weight layout)."""
    x = np.ascontiguousarray(x, dtype=F32)
    mask = np.ascontiguousarray(mask, dtype=F32)

    import ml_dtypes

    BF16 = ml_dtypes.bfloat16
    wqT1 = np.ascontiguousarray(sa_wq.T, dtype=F32)
    wkT1 = np.ascontiguousarray(sa_wk.T.astype(BF16))
    wvT1 = np.ascontiguousarray(sa_wv.T.astype(BF16))
    wqT2 = np.ascontiguousarray(wq.T, dtype=F32)
    wkT2 = np.ascontiguousarray(wk.T.astype(BF16))
    wvT2 = np.ascontiguousarray(wv.T.astype(BF16))

    consts = np.zeros((P, 10), dtype=F32)
    consts[:, 0] = sa_gamma[0]
    consts[:, 1] = gamma[0]
    sgb = (sa_gamma[0] * sa_bv).astype(F32)
    consts[:, 2] = sgb[0:P]
    consts[:, 3] = sgb[P:C]
    consts[0:D, 6] = sa_bq
    consts[0:D, 7] = sa_bk
    consts[0:D, 8] = bq
    consts[0:D, 9] = bk

    in_maps = []
    for g in range(NCORES):
        b, r = g // RSH, g % RSH
        xb = np.ascontiguousarray(x[b].reshape(C, N))
        mb = np.ascontiguousarray(mask[b].reshape(1, N))
        in_maps.append({
            "xf": np.ascontiguousarray(xb.astype(BF16)),
            "xc": np.ascontiguousarray(xb[:, r * R : (r + 1) * R]),
            "mrow": mb,
            "mcrow": np.ascontiguousarray(mb[:, r * R : (r + 1) * R]),
            "wqT1": wqT1, "wkT1": wkT1, "wvT1": wvT1,
            "wqT2": wqT2, "wkT2": wkT2, "wvT2": wvT2,
            "consts": consts,
        })
    return in_maps


def kernel(**inputs):
    from concourse import bass_utils

    if "nc" not in _CACHE:
        _CACHE["nc"] = _build_bass()
    nc = _CACHE["nc"]

    in_maps = _prep_inputs(**inputs)
    res = bass_utils.run_bass_kernel_spmd(
        nc, in_maps, core_ids=list(range(NCORES))
    )
    _CACHE["last_results"] = res

    out = np.empty((B, C, N), dtype=F32)
    for g in range(NCORES):
        b, r = g // RSH, g % RSH
        out[b, :, r * R : (r + 1) * R] = res.results[g]["outc"]
    return out.reshape(B, C, HH, WW)
